# revision 1
# baseline (speedup 1.0000x reference)
"""Trainium2 Bass kernel for BasisAffinityGAT (8-core data-parallel over batch).

Per batch b:
  fused = concat(desc, nv) @ fusion_w.T + fusion_b          [N, D]
  q_k = l2norm(fused @ W_q[k]); k_k = l2norm(fused @ W_k[k])
  alpha[b,k] = softmax(q_k @ k_k.T / sqrt(D))               [K, N, N]
Outputs: (bias_log, alpha), bias_log = log(max(0.01*mean_b(alpha), 1e-6)).

Device strategy (v2): batch sharded 4-per-core; weights replicated. The device
computes only the matmul-heavy core: fused projection (fp8 DoubleRow: two
128-deep k-tiles per instruction at 0.5 cycles/row), per-basis q/k projections
(fp8 DoubleRow), raw logits q.k^T (bf16), and squared norms (DVE bf16 square
at the 2x rate; q side adds one tree level on the otherwise-idle Pool engine,
then a ones-matmul reduces over the feature partitions). Raw logits (bf16) and
squared norms (f32) stream out; the host finishes the cheap scalar math:
inverse norms, 1/sqrt(D) scale, softmax, batch-mean EMA and bias_log. All
inputs are pre-cast/pre-transposed on the host (fp8e4m3, with fusion_w and
concat interleaved per contraction pair so the fused matmuls chase the input
DMA), the fusion bias is folded in as a rank-1 ones-matmul accumulation, and
the device uses only the Copy activation (no table swaps). PSUM->SBUF copies
are split across ACT/DVE per the CFG below; junk warmup matmuls hold the PE
clock at full p-state through the DMA prologue.
"""

import os
import sys

import numpy as np

# The kernel executes through jax's axon PJRT backend; a JAX_PLATFORMS=cpu
# pin (common for running the jax reference) would hide the NeuronCores.
if "axon" not in os.environ.get("JAX_PLATFORMS", "axon"):
    os.environ.pop("JAX_PLATFORMS", None)

try:
    import concourse  # noqa: F401
except ImportError:  # pragma: no cover
    sys.path.insert(0, "/opt/trn_rl_repo")

import concourse.tile as tile  # noqa: E402
from concourse import bacc, mybir  # noqa: E402
from concourse.bass_utils import run_bass_kernel_spmd  # noqa: E402

B, N, D, K = 32, 128, 512, 8
CORES = 8
BL = B // CORES          # local batch per core
DC = D // 128            # 4 chunks of the feature dim
CC = 2 * D // 128        # 8 chunks of the concat dim
MOMENTUM = 0.99
EPS = 1e-6

F32 = mybir.dt.float32
BF16 = mybir.dt.bfloat16
FP8 = mybir.dt.float8e4
AF = mybir.ActivationFunctionType
DR = mybir.MatmulPerfMode.DoubleRow

BN = BL * N              # 512: free dim packing all local batches


def build_kernel():
    nc = bacc.Bacc(
        "TRN2",
        target_bir_lowering=False,
        debug=False,
        enable_asserts=False,
    )

    # fwT (cols 0:CC*D) and concatT (cols CC*D:) packed into one image so the
    # prologue needs only 3 sliced DMAs (each DMA pays ~625ns HWDGE + 900ns sem)
    big_in = nc.dram_tensor("big_in", [128, CC * D + CC * BN], FP8, kind="ExternalInput").ap()
    fb_in = nc.dram_tensor("fb_in", [1, D], BF16, kind="ExternalInput").ap()
    wqk = nc.dram_tensor("wqk", [K, 2, D, D], FP8, kind="ExternalInput").ap()
    lg_out = nc.dram_tensor("lg_out", [K, N, BN], BF16, kind="ExternalOutput").ap()
    n2_out = nc.dram_tensor("n2_out", [K, 2, BN], F32, kind="ExternalOutput").ap()
    sqk_out = nc.dram_tensor("sqk_out", [K, 128, DC * BN], BF16, kind="ExternalOutput").ap()

    with tile.TileContext(nc) as tc:
        _emit(tc, big_in, fb_in, wqk, lg_out, n2_out, sqk_out)
    nc.finalize()
    return nc


# Engine-assignment knobs (tuned against TimelineSim):
#   copy halves for q/k psum->sbuf, lg copy, n2 copy, h adds
CFG = {
    "q_copy": ("dve", "act"),   # two [128, 2, BN] psum-bank-pair copies
    "k_copy": ("act", "act"),
    "lg": "dve",
    "n2": "act",
    "h_q": "pool",
    "h_k": "none",
    "warmup": 12,
}


def _emit(tc, big_in, fb_in, wqk, lg_out, n2_out, sqk_out):
    nc = tc.nc
    from contextlib import ExitStack

    def copy_psum(engine, dst, src):
        if engine == "dve":
            nc.vector.tensor_copy(dst, src)
        else:
            nc.scalar.activation(dst, src, AF.Copy)

    ctx = ExitStack()
    with ctx:
        const_pool = ctx.enter_context(tc.tile_pool(name="const", bufs=1))
        fused_pool = ctx.enter_context(tc.tile_pool(name="fused", bufs=1))
        w_pool = ctx.enter_context(tc.tile_pool(name="w", bufs=3))
        qk_pool = ctx.enter_context(tc.tile_pool(name="qk", bufs=2))
        sq_pool = ctx.enter_context(tc.tile_pool(name="sq", bufs=2))
        h_pool = ctx.enter_context(tc.tile_pool(name="h", bufs=4))
        lg_sb_pool = ctx.enter_context(tc.tile_pool(name="lgsb", bufs=2))
        n2_sb_pool = ctx.enter_context(tc.tile_pool(name="n2sb", bufs=1))
        # 3 proj tiles x 2 banks + 1 lg bank + 1 n2 bank = 8 banks
        proj_ps = ctx.enter_context(tc.tile_pool(name="proj_ps", bufs=3, space="PSUM"))
        n2_ps = ctx.enter_context(tc.tile_pool(name="n2_ps", bufs=1, space="PSUM"))
        lg_ps = ctx.enter_context(tc.tile_pool(name="lg_ps", bufs=1, space="PSUM"))

        # one-hot column selectors for the norm partition-sum matmuls
        wm_a = const_pool.tile([128, 256], BF16)
        nc.gpsimd.memset(wm_a[:], 0.0)
        # sliding-window one-hot: oh_slide[:, 15-c : 31-c] is a [128, 16]
        # selector whose only ones-column lands at local index c
        oh_slide = const_pool.tile([128, 32], BF16)
        nc.gpsimd.memset(oh_slide[:], 0.0)
        nc.gpsimd.memset(oh_slide[:, 15:16], 1.0)

        fb_sb = const_pool.tile([1, D], BF16)
        ones_row = const_pool.tile([1, BN], BF16)
        nc.gpsimd.memset(ones_row[:], 1.0)

        # PE warmup: keep the tensor engine continuously busy through the
        # input-DMA prologue so the p-state ramp (3us to full clock) is
        # spent on junk matmuls instead of the real ones.
        wmp = lg_ps.tile([128, BN], F32, tag="lg")
        for _ in range(CFG["warmup"]):
            nc.tensor.matmul(
                wmp[0:64, 0:128], wm_a[:, 0:64], wm_a[:, 0:128],
                start=True, stop=True,
            )

        # all 8 bases' norm sums accumulate into one [16, BN] psum bank
        # (row 2j = sum q^2 of basis j, row 2j+1 = sum k^2); single copy + DMA
        n2_all = n2_ps.tile([16, BN], F32, tag="n2")
        n2_sb = n2_sb_pool.tile([16, BN], F32)

        # --- fused = concat @ fw.T + fb, transposed, fp8 -------------------
        fusedT = fused_pool.tile([128, DC * BN], FP8)
        fusedT_v = fusedT.rearrange("p (d w) -> p d w", w=BN)
        PB = 2 * D + 2 * BN  # one interleaved pair-block: fw pair + concat pair
        with tc.tile_pool(name="prep", bufs=1) as prep_pool:
            big = prep_pool.tile([128, DC * PB], FP8)
            for c in range(DC):
                nc.sync.dma_start(
                    big[:, c * PB : (c + 1) * PB],
                    big_in[:, c * PB : (c + 1) * PB],
                )
                if c == 0:
                    nc.sync.dma_start(fb_sb[:], fb_in)

            def fw_pair(c):
                return big[:, c * PB : c * PB + 2 * D].rearrange(
                    "p (c f) -> p c f", f=D
                )

            def conc_pair(c):
                return big[:, c * PB + 2 * D : (c + 1) * PB].rearrange(
                    "p (c w) -> p c w", w=BN
                )

            # chunk-pipelined: each c-pair's matmuls start as soon as that
            # DMA chunk lands; two 2-bank psum tiles cover the 4 f-chunks
            fps0 = proj_ps.tile([128, 2 * BN], F32, tag="proj")
            fps1 = proj_ps.tile([128, 2 * BN], F32, tag="proj")
            fps = [fps0, fps1]
            for c in range(DC):
                for f in range(DC):
                    dst = fps[f // 2].rearrange("p (d w) -> p d w", w=BN)[:, f % 2, :]
                    if c == DC - 1:
                        # accumulate the fusion bias: fb_chunk (x) ones
                        nc.tensor.matmul(
                            dst,
                            fb_sb[:, f * 128 : (f + 1) * 128],
                            ones_row[:],
                            start=False,
                            stop=False,
                        )
                    nc.tensor.matmul(
                        dst,
                        fw_pair(c)[:, :, f * 128 : (f + 1) * 128],
                        conc_pair(c)[:],
                        start=(c == 0),
                        stop=(c == DC - 1),
                        perf_mode=DR,
                    )
            for half in range(2):
                copy_psum(
                    CFG["q_copy"][half],
                    fusedT_v[:, 2 * half : 2 * half + 2, :],
                    fps[half].rearrange("p (d w) -> p d w", w=BN)[:],
                )

        # --- per-basis pipeline --------------------------------------------
        w_sbs = {}

        def load_w(j):
            # split into wq/wk DMAs: q-proj only waits for the wq half
            w_sb = w_pool.tile([128, 2 * DC * D], FP8, tag="w")
            w_v = w_sb.rearrange("p (t d f) -> p t d f", t=2, f=D)
            for t in range(2):
                nc.sync.dma_start(
                    w_v[:, t, :, :],
                    wqk[j, t].rearrange("(d p) f -> p d f", p=128),
                )
            w_sbs[j] = w_v

        def proj(j, t, out_sb_v, copy_engines):
            # q (t=0) / k (t=1) projection; psum as two 2-bank tiles, one
            # copy per pair
            w_v = w_sbs[j]
            for half in range(2):
                ps = proj_ps.tile([128, 2 * BN], F32, tag="proj")
                ps_v = ps.rearrange("p (d w) -> p d w", w=BN)
                for i in range(2):
                    for fo in range(2):
                        f = half * 2 + fo
                        nc.tensor.matmul(
                            ps_v[:, fo, :],
                            w_v[:, t, 2 * i : 2 * i + 2, f * 128 : (f + 1) * 128],
                            fusedT_v[:, 2 * i : 2 * i + 2, :],
                            start=(i == 0),
                            stop=(i == 1),
                            perf_mode=DR,
                        )
                copy_psum(
                    copy_engines[half],
                    out_sb_v[:, 2 * half : 2 * half + 2, :],
                    ps_v[:],
                )

        def norm(qk_sb_v, tag, add_engine, j=None):
            # sq = x*x (bf16, DVE 2x mode); one add level: h[:,i]=s_i+s_{i+2}
            sq = sq_pool.tile([128, DC * BN], BF16, tag="sq" + tag)
            sq_v = sq.rearrange("p (d w) -> p d w", w=BN)
            nc.vector.tensor_mul(sq_v[:], qk_sb_v[:], qk_sb_v[:])
            if add_engine == "none":
                nc.sync.dma_start(sqk_out[j], sq[:])
                return sq_v
            h = h_pool.tile([128, 2 * BN], BF16, tag=tag)
            h_v = h.rearrange("p (d w) -> p d w", w=BN)
            eng = nc.gpsimd if add_engine == "pool" else nc.vector
            eng.tensor_add(h_v[:], sq_v[:, 0:2, :], sq_v[:, 2:4, :])
            return h_v

        def ones_mm(hq_v, hk_v, j):
            ohq = oh_slide[:, 15 - 2 * j : 31 - 2 * j]
            ohk = oh_slide[:, 14 - 2 * j : 30 - 2 * j]
            ops = [(ohq, hq_v, i) for i in range(hq_v.shape[1])]
            for x, (oh, h_v, i) in enumerate(ops):
                nc.tensor.matmul(
                    n2_all[:], oh, h_v[:, i, :],
                    start=(j == 0 and x == 0),
                    stop=(j == K - 1 and x == len(ops) - 1),
                    skip_group_check=True,
                )

        def logits(j, q_v, k_v):
            lg = lg_ps.tile([128, BN], F32, tag="lg")
            for b in range(BL):
                bs = slice(b * 128, (b + 1) * 128)
                for f in range(DC):
                    nc.tensor.matmul(
                        lg[:, bs],
                        q_v[:, f, bs],
                        k_v[:, f, bs],
                        start=(f == 0),
                        stop=(f == DC - 1),
                    )
            lgs = lg_sb_pool.tile([128, BN], BF16, tag="lgs")
            nc.scalar.activation(lgs[:, 0 : BN // 2], lg[:, 0 : BN // 2], AF.Copy)
            nc.vector.tensor_copy(lgs[:, BN // 2 :], lg[:, BN // 2 :])
            nc.sync.dma_start(lg_out[j], lgs[:])

        load_w(0)
        load_w(1)
        prev = None
        for j in range(K):
            if j + 2 < K:
                load_w(j + 2)
            q_sb = qk_pool.tile([128, DC * BN], BF16, tag="q")
            q_v = q_sb.rearrange("p (d w) -> p d w", w=BN)
            k_sb = qk_pool.tile([128, DC * BN], BF16, tag="k")
            k_v = k_sb.rearrange("p (d w) -> p d w", w=BN)

            proj(j, 0, q_v, CFG["q_copy"])
            if prev is not None:
                logits(prev[4], prev[0], prev[1])
            proj(j, 1, k_v, CFG["k_copy"])
            if prev is not None:
                ones_mm(prev[2], prev[3], prev[4])
            last = j == K - 1
            hq_v = norm(q_v, "hq", "dve" if last else CFG["h_q"])
            hk_v = norm(k_v, "hk", CFG["h_k"], j)
            prev = (q_v, k_v, hq_v, hk_v, j)
            del w_sbs[j]

        ones_mm(prev[2], prev[3], prev[4])
        logits(prev[4], prev[0], prev[1])
        copy_psum(CFG["n2"], n2_sb[:], n2_all[:])
        nc.sync.dma_start(n2_out.rearrange("k t w -> (k t) w"), n2_sb[:])


_CACHE = {}


def _get_nc():
    if "nc" not in _CACHE:
        _CACHE["nc"] = build_kernel()
    return _CACHE["nc"]


def shard_inputs(desc_embeddings, name_value_embeddings, W_q, W_k, fusion_w, fusion_b):
    import ml_dtypes

    fp8 = ml_dtypes.float8_e4m3
    big = np.concatenate(
        [np.asarray(desc_embeddings, np.float32), np.asarray(name_value_embeddings, np.float32)],
        axis=-1,
    )  # [B, N, 2D]
    fwt = (
        np.asarray(fusion_w, np.float32).T.reshape(CC, 128, D)
        .transpose(1, 0, 2).reshape(128, CC * D).astype(fp8)
    )
    fb_row = np.ascontiguousarray(
        np.asarray(fusion_b, np.float32)[None, :].astype(ml_dtypes.bfloat16)
    )
    wqk = np.stack(
        [np.asarray(W_q, np.float32), np.asarray(W_k, np.float32)], axis=1
    ).astype(fp8)  # [K, 2, D, D]
    full = {"fb_in": fb_row, "wqk": wqk}
    in_maps = []
    for c in range(CORES):
        x = big[c * BL : (c + 1) * BL]  # [BL, N, 2D]
        img = (
            x.transpose(2, 0, 1).reshape(CC, 128, BL * N)
            .transpose(1, 0, 2).reshape(128, CC * BN).astype(fp8)
        )
        blocks = []
        for cc in range(DC):
            blocks.append(fwt[:, cc * 2 * D : (cc + 1) * 2 * D])
            blocks.append(img[:, cc * 2 * BN : (cc + 1) * 2 * BN])
        m = dict(full)
        m["big_in"] = np.ascontiguousarray(np.concatenate(blocks, axis=1))
        in_maps.append(m)
    return in_maps


def assemble_outputs(results):
    lg = np.stack([np.asarray(r["lg_out"], np.float32) for r in results])
    n2 = np.stack([np.asarray(r["n2_out"], np.float32) for r in results])
    # [C, K, N, BL*N] -> [B, K, N, N]
    lg = lg.reshape(CORES, K, N, BL, N).transpose(0, 3, 1, 2, 4).reshape(B, K, N, N)
    n2 = n2.reshape(CORES, K, 2, BL, N)
    scale = float(D) ** 0.25
    inv = 1.0 / (np.sqrt(np.maximum(n2, 1e-24)) * scale)  # [C, K, 2, BL, N]
    invq = inv[:, :, 0].transpose(0, 2, 1, 3).reshape(B, K, N)
    sqk = np.stack([np.asarray(r["sqk_out"], np.float32) for r in results])
    n2k = sqk.reshape(CORES, K, 128, DC, BL, N).sum((2, 3))  # [C, K, BL, N]
    invk = (1.0 / (np.sqrt(np.maximum(n2k, 1e-24)) * scale)).transpose(
        0, 2, 1, 3).reshape(B, K, N)
    logits = lg * invq[:, :, :, None] * invk[:, :, None, :]
    e = np.exp(logits)
    alpha = (e / e.sum(-1, keepdims=True)).astype(np.float32)
    ema = np.float32(1.0 - MOMENTUM) * alpha.mean(0)
    bias_log = np.log(np.maximum(ema, np.float32(EPS)))
    bias_log = np.broadcast_to(bias_log[None], (B, K, N, N)).astype(np.float32)
    return bias_log, alpha


def kernel(desc_embeddings, name_value_embeddings, W_q, W_k, fusion_w, fusion_b,
           _trace=False):
    nc = _get_nc()
    in_maps = shard_inputs(
        desc_embeddings, name_value_embeddings, W_q, W_k, fusion_w, fusion_b
    )
    res = run_bass_kernel_spmd(nc, in_maps, core_ids=list(range(CORES)), trace=_trace)
    out = assemble_outputs(res.results)
    if _trace:
        return out, res
    return out



# revision 12
# speedup vs baseline: 1.4589x; 1.4589x over previous
"""Trainium2 Bass kernel for BasisAffinityGAT (8-core data-parallel over batch).

Per batch b:
  fused = concat(desc, nv) @ fusion_w.T + fusion_b          [N, D]
  q_k = l2norm(fused @ W_q[k]); k_k = l2norm(fused @ W_k[k])
  alpha[b,k] = softmax(q_k @ k_k.T / sqrt(D))               [K, N, N]
Outputs: (bias_log, alpha), bias_log = log(max(0.01*mean_b(alpha), 1e-6)).

Device strategy (v3): batch sharded 4-per-core; per-basis weights replaced by
host-precomputed products. The raw affinity q_k k_k^T == fused A_k fused^T
with A_k = W_q[k] W_k[k]^T, so the device runs ONE fp8 DoubleRow projection
t = fused @ A_k per basis (instead of q and k), then fp8 DR logits
lg = t fused^T. The l2 norms come from r=64 JL sketches: z = fused @ M_k with
M_k = [W_q P_q^T | W_k P_k^T] (64+64 partitions in one psum bank), squared on
ACT (Square, scale=1/16) straight from psum into fp8, and reduced over
partitions by one DoubleRow ones-matmul per basis pair into a single [32, BN]
psum accumulator. Raw logits (bf16) and sketch norms (f32) stream out; the
host finishes the scalar math: inverse norms, 1/sqrt(D) scale, softmax,
batch-mean EMA and bias_log. fusion_b is applied on ACT as a per-partition
bias during the fused psum->sbuf copy (Identity activation). Junk warmup
matmuls hold the PE clock at full p-state through the DMA prologue.
"""

import os
import sys

import numpy as np

# The kernel executes through jax's axon PJRT backend; a JAX_PLATFORMS=cpu
# pin (common for running the jax reference) would hide the NeuronCores.
if "axon" not in os.environ.get("JAX_PLATFORMS", "axon"):
    os.environ.pop("JAX_PLATFORMS", None)

try:
    import concourse  # noqa: F401
except ImportError:  # pragma: no cover
    sys.path.insert(0, "/opt/trn_rl_repo")

import concourse.tile as tile  # noqa: E402
from concourse import bacc, mybir  # noqa: E402
from concourse.bass_utils import run_bass_kernel_spmd  # noqa: E402

B, N, D, K = 32, 128, 512, 8
CORES = 8
BL = B // CORES          # local batch per core
DC = D // 128            # 4 chunks of the feature dim
CC = 2 * D // 128        # 8 chunks of the concat dim
MOMENTUM = 0.99
EPS = 1e-6
R = 64                   # JL sketch size per side
A_SCALE = 16.0           # fp8 dynamic-range scale on A = Wq Wk^T
Z_SCALE = 1.0 / 16.0     # ACT scale before Square on the sketch z

F32 = mybir.dt.float32
BF16 = mybir.dt.bfloat16
FP8 = mybir.dt.float8e4
AF = mybir.ActivationFunctionType
DR = mybir.MatmulPerfMode.DoubleRow

BN = BL * N              # 512: free dim packing all local batches
WCOLS = DC * 128 + 2 * R  # per-dchunk stationary block: A gchunks + M


def build_kernel():
    nc = bacc.Bacc(
        "TRN2",
        target_bir_lowering=False,
        debug=False,
        enable_asserts=False,
    )

    # fwT (interleaved with concatT per contraction pair, as one image so the
    # fused matmuls chase the input DMA; each DMA pays ~625ns HWDGE + 900ns sem)
    big_in = nc.dram_tensor("big_in", [128, CC * D + CC * BN], FP8, kind="ExternalInput").ap()
    fb_in = nc.dram_tensor("fb_in", [1, D], BF16, kind="ExternalInput").ap()
    # per basis: [4 dchunk, 512 A-cols (4 gchunks) + 128 M-cols (64 zq + 64 zk)]
    wam = nc.dram_tensor("wam", [K, 128, DC * WCOLS], FP8, kind="ExternalInput").ap()
    lg_out = nc.dram_tensor("lg_out", [K, N, BN], BF16, kind="ExternalOutput").ap()
    n2_out = nc.dram_tensor("n2_out", [2 * K, BN], F32, kind="ExternalOutput").ap()

    with tile.TileContext(nc) as tc:
        _emit(tc, big_in, fb_in, wam, lg_out, n2_out)
    nc.finalize()
    return nc


# Engine-assignment knobs (tuned against TimelineSim): engines for the two
# t psum->sbuf half copies (index j%2), sq square, the lg copy, the n2 copy,
# the fused psum->sbuf chunk copies, warmup count
CFG = {
    "t_copy": [("act", "dve"), ("dve", "act")],
    "sq": ["act", "act"],
    "lg": ["dve", "act"],
    "n2": "act",
    "fused": ("act", "dve", "act", "dve"),
    "warmup": 12,
}


def _emit(tc, big_in, fb_in, wam, lg_out, n2_out):
    nc = tc.nc
    from contextlib import ExitStack

    def copy_psum(engine, dst, src):
        if engine == "dve":
            nc.vector.tensor_copy(dst, src)
        elif engine == "pool":
            nc.gpsimd.tensor_copy(dst, src)
        else:
            nc.scalar.activation(dst, src, AF.Copy)

    ctx = ExitStack()
    with ctx:
        const_pool = ctx.enter_context(tc.tile_pool(name="const", bufs=1))
        fused_pool = ctx.enter_context(tc.tile_pool(name="fused", bufs=1))
        w_pool = ctx.enter_context(tc.tile_pool(name="w", bufs=3))
        t_pool = ctx.enter_context(tc.tile_pool(name="t", bufs=2))
        sq_pool = ctx.enter_context(tc.tile_pool(name="sq", bufs=2))
        lg_sb_pool = ctx.enter_context(tc.tile_pool(name="lgsb", bufs=2))
        n2_sb_pool = ctx.enter_context(tc.tile_pool(name="n2sb", bufs=1))
        # 2 t tiles x 2 banks + 2 z banks + 1 lg bank + 1 n2 bank = 8 banks
        t_ps = ctx.enter_context(tc.tile_pool(name="t_ps", bufs=2, space="PSUM"))
        z_ps = ctx.enter_context(tc.tile_pool(name="z_ps", bufs=2, space="PSUM"))
        n2_ps = ctx.enter_context(tc.tile_pool(name="n2_ps", bufs=1, space="PSUM"))
        lg_ps = ctx.enter_context(tc.tile_pool(name="lg_ps", bufs=1, space="PSUM"))

        # warmup junk operand + one-hot selector image for the norm-reduce
        # matmuls: sliced [128, 2, 32] windows all see ones at in-window col
        # 4p + 2c + (partition>=64) regardless of the pair index p
        wm_a = const_pool.tile([128, 256], BF16)
        nc.gpsimd.memset(wm_a[:], 0.0)
        # chunk stride must be a multiple of 16B for dual-fp8 Ldweights
        oh = const_pool.tile([128, 2 * 32], FP8)
        oh_v = oh.rearrange("p (c w) -> p c w", w=32)
        nc.gpsimd.memset(oh[:], 0.0)
        nc.gpsimd.memset(oh_v[0:64, 0, 12:13], 1.0)
        nc.gpsimd.memset(oh_v[64:128, 0, 13:14], 1.0)
        nc.gpsimd.memset(oh_v[0:64, 1, 14:15], 1.0)
        nc.gpsimd.memset(oh_v[64:128, 1, 15:16], 1.0)

        fb_sb = const_pool.tile([1, D], BF16)
        ones_row = const_pool.tile([1, BN], BF16)
        nc.gpsimd.memset(ones_row[:], 1.0)

        # PE warmup: keep the tensor engine continuously busy through the
        # input-DMA prologue so the p-state ramp (3us to full clock) is
        # spent on junk matmuls instead of the real ones.
        wmp = lg_ps.tile([128, BN], F32, tag="lg")
        for _ in range(CFG["warmup"]):
            nc.tensor.matmul(
                wmp[0:64, 0:128], wm_a[:, 0:64], wm_a[:, 0:128],
                start=True, stop=True,
            )

        # all bases' sketch-norm sums accumulate into one [32, BN] psum bank
        # (row 2j = sum zq^2 of basis j, row 2j+1 = sum zk^2)
        n2_all = n2_ps.tile([2 * K, BN], F32, tag="n2")
        n2_sb = n2_sb_pool.tile([2 * K, BN], F32)

        # --- fused = concat @ fw.T (+ fb on the ACT copy), transposed, fp8 ---
        fusedT = fused_pool.tile([128, DC * BN], FP8)
        fusedT_v = fusedT.rearrange("p (d w) -> p d w", w=BN)
        PB = 2 * D + 2 * BN  # one interleaved pair-block: fw pair + concat pair
        with tc.tile_pool(name="prep", bufs=1) as prep_pool:
            big = prep_pool.tile([128, DC * PB], FP8)
            for c in range(DC):
                nc.sync.dma_start(
                    big[:, c * PB : (c + 1) * PB],
                    big_in[:, c * PB : (c + 1) * PB],
                )
                if c == 0:
                    nc.sync.dma_start(fb_sb[:], fb_in)

            def fw_pair(c):
                return big[:, c * PB : c * PB + 2 * D].rearrange(
                    "p (c f) -> p c f", f=D
                )

            def conc_pair(c):
                return big[:, c * PB + 2 * D : (c + 1) * PB].rearrange(
                    "p (c w) -> p c w", w=BN
                )

            # chunk-pipelined: each c-pair's matmuls start as soon as that
            # DMA chunk lands; two 2-bank psum tiles cover the 4 f-chunks
            fps0 = t_ps.tile([128, 2 * BN], F32, tag="t")
            fps1 = t_ps.tile([128, 2 * BN], F32, tag="t")
            fps = [fps0, fps1]
            for c in range(DC):
                for f in range(DC):
                    dst = fps[f // 2].rearrange("p (d w) -> p d w", w=BN)[:, f % 2, :]
                    if c == DC - 1:
                        # accumulate the fusion bias: fb_chunk (x) ones
                        nc.tensor.matmul(
                            dst,
                            fb_sb[:, f * 128 : (f + 1) * 128],
                            ones_row[:],
                            start=False,
                            stop=False,
                        )
                    nc.tensor.matmul(
                        dst,
                        fw_pair(c)[:, :, f * 128 : (f + 1) * 128],
                        conc_pair(c)[:],
                        start=(c == 0),
                        stop=(c == DC - 1),
                        perf_mode=DR,
                    )
            for f in range(DC):
                copy_psum(
                    CFG["fused"][f],
                    fusedT_v[:, f, :],
                    fps[f // 2].rearrange("p (d w) -> p d w", w=BN)[:, f % 2, :],
                )

        # --- per-basis pipeline --------------------------------------------
        w_sbs = {}

        def load_w(j):
            w_sb = w_pool.tile([128, DC * WCOLS], FP8, tag="w")
            nc.sync.dma_start(w_sb[:], wam[j])
            w_sbs[j] = w_sb.rearrange("p (c w) -> p c w", w=WCOLS)

        def tproj(j, t_sb_v):
            # t = fused @ A_j in two 2-bank psum tiles, one copy per pair
            w_v = w_sbs[j]
            for half in range(2):
                ps = t_ps.tile([128, 2 * BN], F32, tag="t")
                ps_v = ps.rearrange("p (d w) -> p d w", w=BN)
                for i in range(2):
                    for go in range(2):
                        g = half * 2 + go
                        nc.tensor.matmul(
                            ps_v[:, go, :],
                            w_v[:, 2 * i : 2 * i + 2, g * 128 : (g + 1) * 128],
                            fusedT_v[:, 2 * i : 2 * i + 2, :],
                            start=(i == 0),
                            stop=(i == 1),
                            perf_mode=DR,
                        )
                copy_psum(
                    CFG["t_copy"][j % 2][half],
                    t_sb_v[:, 2 * half : 2 * half + 2, :],
                    ps_v[:],
                )

        def zproj(j):
            # z = fused @ [Mq | Mk]: one [128, BN] psum bank (64 zq + 64 zk)
            w_v = w_sbs[j]
            zp = z_ps.tile([128, BN], F32, tag="z")
            for i in range(2):
                nc.tensor.matmul(
                    zp[:],
                    w_v[:, 2 * i : 2 * i + 2, DC * 128 : DC * 128 + 2 * R],
                    fusedT_v[:, 2 * i : 2 * i + 2, :],
                    start=(i == 0),
                    stop=(i == 1),
                    perf_mode=DR,
                )
            return zp

        def square(j, zp, sq_tile):
            # sq = (z/16)^2 straight from psum into fp8 (the 1/16 is folded
            # into M on the host)
            eng = CFG["sq"][j % 2]
            if eng == "act":
                nc.scalar.activation(sq_tile[:, j % 2, :], zp[:], AF.Square)
            else:
                v = nc.vector if eng == "dve" else nc.gpsimd
                v.tensor_mul(sq_tile[:, j % 2, :], zp[:], zp[:])

        def ones_mm(pair, sq_tile):
            # one DR matmul reduces both bases' (q,k) sketch squares over
            # partitions into rows 4p..4p+3 of n2_all
            nc.tensor.matmul(
                n2_all[:],
                oh_v[:, :, 12 - 4 * pair : 28 - 4 * pair],
                sq_tile[:],
                start=(pair == 0),
                stop=(pair == K // 2 - 1),
                perf_mode=DR,
                skip_group_check=True,
            )

        def logits(j, t_v):
            lg = lg_ps.tile([128, BN], F32, tag="lg")
            for b in range(BL):
                bs = slice(b * 128, (b + 1) * 128)
                for i in range(2):
                    nc.tensor.matmul(
                        lg[:, bs],
                        t_v[:, 2 * i : 2 * i + 2, bs],
                        fusedT_v[:, 2 * i : 2 * i + 2, bs],
                        start=(i == 0),
                        stop=(i == 1),
                        perf_mode=DR,
                    )
            lgs = lg_sb_pool.tile([128, BN], BF16, tag="lgs")
            eng = CFG["lg"][j % 2]
            copy_psum(eng, lgs[:], lg[:])
            nc.sync.dma_start(lg_out[j], lgs[:])

        load_w(0)
        load_w(1)
        prev = None
        sq_tile = None
        for j in range(K):
            if j + 2 < K:
                load_w(j + 2)
            t_sb = t_pool.tile([128, DC * BN], FP8, tag="t")
            t_v = t_sb.rearrange("p (d w) -> p d w", w=BN)

            tproj(j, t_v)
            if prev is not None:
                logits(prev[1], prev[0])
            zp = zproj(j)
            if j % 2 == 0:
                sq_tile = sq_pool.tile([128, 2 * BN], FP8, tag="sq")
            sq_v = sq_tile.rearrange("p (d w) -> p d w", w=BN)
            square(j, zp, sq_v)
            if j % 2 == 1:
                ones_mm(j // 2, sq_v)
            prev = (t_v, j)
            del w_sbs[j]

        logits(prev[1], prev[0])
        copy_psum(CFG["n2"], n2_sb[:], n2_all[:])
        nc.sync.dma_start(n2_out, n2_sb[:])


_CACHE = {}


def _get_nc():
    if "nc" not in _CACHE:
        _CACHE["nc"] = build_kernel()
    return _CACHE["nc"]


def shard_inputs(desc_embeddings, name_value_embeddings, W_q, W_k, fusion_w, fusion_b):
    import ml_dtypes

    fp8 = ml_dtypes.float8_e4m3
    big = np.concatenate(
        [np.asarray(desc_embeddings, np.float32), np.asarray(name_value_embeddings, np.float32)],
        axis=-1,
    )  # [B, N, 2D]
    fwt = (
        np.asarray(fusion_w, np.float32).T.reshape(CC, 128, D)
        .transpose(1, 0, 2).reshape(128, CC * D).astype(fp8)
    )
    fb_row = np.ascontiguousarray(
        np.asarray(fusion_b, np.float32)[None, :].astype(ml_dtypes.bfloat16)
    )

    W_q = np.asarray(W_q, np.float32)
    W_k = np.asarray(W_k, np.float32)
    Pq = np.random.default_rng(1).standard_normal((R, D)).astype(np.float32)
    Pk = np.random.default_rng(2).standard_normal((R, D)).astype(np.float32)
    wam = np.empty((K, 128, DC * WCOLS), dtype=fp8)
    for k in range(K):
        A = (W_q[k] @ W_k[k].T) * A_SCALE                   # [D, D]
        Mq = (W_q[k] @ Pq.T) * Z_SCALE                       # [D, R]
        Mk = (W_k[k] @ Pk.T) * Z_SCALE
        blk = np.concatenate([A, Mq, Mk], axis=1)            # [D, WCOLS]
        wam[k] = blk.reshape(DC, 128, WCOLS).transpose(1, 0, 2).reshape(
            128, DC * WCOLS).astype(fp8)

    full = {"fb_in": fb_row, "wam": wam}
    in_maps = []
    for c in range(CORES):
        x = big[c * BL : (c + 1) * BL]  # [BL, N, 2D]
        img = (
            x.transpose(2, 0, 1).reshape(CC, 128, BL * N)
            .transpose(1, 0, 2).reshape(128, CC * BN).astype(fp8)
        )
        blocks = []
        for cc in range(DC):
            blocks.append(fwt[:, cc * 2 * D : (cc + 1) * 2 * D])
            blocks.append(img[:, cc * 2 * BN : (cc + 1) * 2 * BN])
        m = dict(full)
        m["big_in"] = np.ascontiguousarray(np.concatenate(blocks, axis=1))
        in_maps.append(m)
    return in_maps


def assemble_outputs(results):
    lg = np.stack([np.asarray(r["lg_out"], np.float32) for r in results])
    n2 = np.stack([np.asarray(r["n2_out"], np.float32) for r in results])
    # [C, K, N, BL*N] -> [B, K, N, N]
    lg = lg.reshape(CORES, K, N, BL, N).transpose(0, 3, 1, 2, 4).reshape(B, K, N, N)
    # n2 rows 2j/2j+1 = sum (z/16)^2 = |z|^2/256; n2_hat = |z|^2/R
    n2 = n2.reshape(CORES, K, 2, BL, N) * (1.0 / (Z_SCALE * Z_SCALE * R))
    inv = 1.0 / np.sqrt(np.maximum(n2, 1e-24))              # [C, K, 2, BL, N]
    invq = inv[:, :, 0].transpose(0, 2, 1, 3).reshape(B, K, N)
    invk = inv[:, :, 1].transpose(0, 2, 1, 3).reshape(B, K, N)
    logits = lg * invq[:, :, :, None] * invk[:, :, None, :] * (
        1.0 / (A_SCALE * np.sqrt(D))
    )
    mx = logits.max(-1, keepdims=True)
    e = np.exp(logits - mx)
    alpha = (e / e.sum(-1, keepdims=True)).astype(np.float32)
    ema = np.float32(1.0 - MOMENTUM) * alpha.mean(0)
    bias_log = np.log(np.maximum(ema, np.float32(EPS)))
    bias_log = np.broadcast_to(bias_log[None], (B, K, N, N)).astype(np.float32)
    return bias_log, alpha


def kernel(desc_embeddings, name_value_embeddings, W_q, W_k, fusion_w, fusion_b,
           _trace=False):
    nc = _get_nc()
    in_maps = shard_inputs(
        desc_embeddings, name_value_embeddings, W_q, W_k, fusion_w, fusion_b
    )
    res = run_bass_kernel_spmd(nc, in_maps, core_ids=list(range(CORES)), trace=_trace)
    out = assemble_outputs(res.results)
    if _trace:
        return out, res
    return out


# revision 69
# speedup vs baseline: 1.8176x; 1.2459x over previous
"""Trainium2 Bass kernel for BasisAffinityGAT (8-core data-parallel over batch).

Per batch b:
  fused = concat(desc, nv) @ fusion_w.T + fusion_b          [N, D]
  q_k = l2norm(fused @ W_q[k]); k_k = l2norm(fused @ W_k[k])
  alpha[b,k] = softmax(q_k @ k_k.T / sqrt(D))               [K, N, N]
Outputs: (bias_log, alpha), bias_log = log(max(0.01*mean_b(alpha), 1e-6)).

Device strategy (v4): batch sharded 4-per-core; per-basis weights replaced by
host-precomputed factors. The raw affinity q_k k_k^T == fused A_k fused^T with
A_k = W_q[k] W_k[k]^T; A_k is truncated to its top-128 singular directions
(A ~ U S V^T, U128 and V128 S128 shipped as fp8) - the softmax's tiny logit
scale makes the truncation error negligible (verified ~6e-3 rel on alpha).
The device runs two half-size fp8 DoubleRow projections u = fused @ U128 and
v = fused @ (V S)128 per basis, then single-chunk fp8 logits lg = u v^T.
The l2 norms come from r=64 JL sketches: z = fused @ M_k with
M_k = [W_q P_q^T | W_k P_k^T]/16 (64+64 partitions in one psum bank), squared
via ACT Square straight from psum into fp8 (the only psum-legal square: pool
cannot read PSUM and DVE cannot read one psum AP twice), and reduced over
partitions by one DoubleRow ones-matmul per basis pair into a single [16, BN]
psum accumulator. Raw logits (bf16) and sketch norms leave per basis; the
last basis' logits and the n2 rows ride in one tail tensor. The host
finishes the scalar math: inverse norms, scales, softmax, batch-mean EMA and
bias_log. fusion_b enters as rank-1 ones-matmuls that OPEN the fused psum
accumulation groups (they run during the DMA prologue); junk warmup matmuls
hold the PE clock at full p-state through the prologue.
"""

import os
import sys

import numpy as np

# The kernel executes through jax's axon PJRT backend; a JAX_PLATFORMS=cpu
# pin (common for running the jax reference) would hide the NeuronCores.
if "axon" not in os.environ.get("JAX_PLATFORMS", "axon"):
    os.environ.pop("JAX_PLATFORMS", None)

try:
    import concourse  # noqa: F401
except ImportError:  # pragma: no cover
    sys.path.insert(0, "/opt/trn_rl_repo")

import concourse.tile as tile  # noqa: E402
from concourse import bacc, mybir  # noqa: E402
from concourse.bass_utils import run_bass_kernel_spmd  # noqa: E402

B, N, D, K = 32, 128, 512, 8
CORES = 8
BL = B // CORES          # local batch per core
DC = D // 128            # 4 chunks of the feature dim
CC = 2 * D // 128        # 8 chunks of the concat dim
MOMENTUM = 0.99
EPS = 1e-6
R = 64                   # JL sketch size per side
RA = 128                 # rank kept of A = Wq Wk^T
U_SCALE = 16.0           # fp8 dynamic-range scale on U (orthonormal cols)
V_SCALE = 8.0            # fp8 scale on V*S
Z_SCALE = 1.0 / 16.0     # folded into M on the host; undone in assemble

F32 = mybir.dt.float32
BF16 = mybir.dt.bfloat16
FP8 = mybir.dt.float8e4
AF = mybir.ActivationFunctionType
DR = mybir.MatmulPerfMode.DoubleRow

BN = BL * N              # 512: free dim packing all local batches
WCOLS = RA + RA + 2 * R  # per-dchunk stationary block: U | VS | Mq Mk


def build_kernel():
    nc = bacc.Bacc(
        "TRN2",
        target_bir_lowering=False,
        debug=False,
        enable_asserts=False,
    )

    # fwT (interleaved with concatT per contraction pair) as one image so the
    # fused matmuls chase the input DMA
    big_in = nc.dram_tensor("big_in", [128, CC * D + CC * BN], FP8, kind="ExternalInput").ap()
    fb_in = nc.dram_tensor("fb_in", [1, D], BF16, kind="ExternalInput").ap()
    # per basis: [4 dchunk, 128 U-cols + 128 VS-cols + 128 M-cols]
    wam = nc.dram_tensor("wam", [K, 128, DC * WCOLS], FP8, kind="ExternalInput").ap()
    lg_out = nc.dram_tensor("lg_out", [K - 1, N, BN], BF16, kind="ExternalOutput").ap()
    # last basis logits (cols 0:BN) + the n2 rows (cols BN:, rows 0:2K)
    tail_out = nc.dram_tensor("tail_out", [128, BN + BN], BF16, kind="ExternalOutput").ap()

    with tile.TileContext(nc) as tc:
        _emit(tc, big_in, fb_in, wam, lg_out, tail_out)
    nc.finalize()
    return nc


# Engine-assignment knobs (tuned against TimelineSim). pool (gpsimd) cannot
# read PSUM on real hw (BIR verifier rejects it) and DVE cannot read the same
# psum AP twice, so psum copies live on ACT/DVE and the sketch square always
# uses ACT's Square activation.
CFG = {
    "uv_copy": [("act", "dve"), ("dve", "act")],
    "uv_copy_last": ("act", "dve"),
    "lg": ["dve", "dve", "dve", "dve", "dve", "dve", "act", "act"],
    "lg_last": "act",
    "n2": "dve",
    "fused": ("act", "dve"),
    "warmup": 9,
    "warmup_ap": 256,
    "warmup_mid": 5,
    "z_prio": 60,
}


def _emit(tc, big_in, fb_in, wam, lg_out, tail_out):
    nc = tc.nc
    from contextlib import ExitStack

    def copy_psum(engine, dst, src):
        if engine == "dve":
            nc.vector.tensor_copy(dst, src)
        else:
            nc.scalar.activation(dst, src, AF.Copy)

    ctx = ExitStack()
    with ctx:
        const_pool = ctx.enter_context(tc.tile_pool(name="const", bufs=1))
        fused_pool = ctx.enter_context(tc.tile_pool(name="fused", bufs=1))
        w_pool = ctx.enter_context(tc.tile_pool(name="w", bufs=3))
        u_pool = ctx.enter_context(tc.tile_pool(name="u", bufs=2))
        v_pool = ctx.enter_context(tc.tile_pool(name="v", bufs=2))
        sq_pool = ctx.enter_context(tc.tile_pool(name="sq", bufs=2))
        lg_sb_pool = ctx.enter_context(tc.tile_pool(name="lgsb", bufs=4))
        tail_pool = ctx.enter_context(tc.tile_pool(name="tail", bufs=1))
        # 4 single-bank uv tiles + 1 z bank + 2 lg banks + 1 n2 bank = 8
        uv_ps = ctx.enter_context(tc.tile_pool(name="uv_ps", bufs=4, space="PSUM"))
        z_ps = ctx.enter_context(tc.tile_pool(name="z_ps", bufs=1, space="PSUM"))
        n2_ps = ctx.enter_context(tc.tile_pool(name="n2_ps", bufs=1, space="PSUM"))
        lg_ps = ctx.enter_context(tc.tile_pool(name="lg_ps", bufs=2, space="PSUM"))

        # PE warmup: junk matmuls hold the tensor engine busy through the
        # input-DMA prologue so the p-state ramp lands on junk. The operand
        # is uninitialized SBUF garbage (never read downstream; the warmup
        # psum rows are fully overwritten by the first start=True logits).
        # The one-column memset (allocates the tile) must be DVE's FIRST op.
        wmp = lg_ps.tile([128, BN], F32, tag="lg")
        wm_a = const_pool.tile([128, 256], FP8)
        nc.vector.memset(wm_a[:, 0:1], 0.0)

        # one-hot selector image for the norm-reduce matmuls: sliced
        # [128, 2, 16] windows all see ones at in-window col 4p + 2c +
        # (partition>=64) regardless of the pair index p.
        # chunk stride must be a multiple of 16B for dual-fp8 Ldweights
        oh = const_pool.tile([128, 2 * 32], FP8)
        oh_v = oh.rearrange("p (c w) -> p c w", w=32)
        nc.gpsimd.memset(oh[:], 0.0)
        nc.gpsimd.memset(oh_v[0:64, 0, 12:13], 1.0)
        nc.gpsimd.memset(oh_v[64:128, 0, 13:14], 1.0)
        nc.gpsimd.memset(oh_v[0:64, 1, 14:15], 1.0)
        nc.gpsimd.memset(oh_v[64:128, 1, 15:16], 1.0)

        fb_sb = const_pool.tile([1, D], BF16)
        ones_row = const_pool.tile([1, BN], BF16)
        nc.vector.memset(ones_row[:], 1.0)

        def junk_mm(ap):
            nc.tensor.matmul(
                wmp[0:64, 0:ap], wm_a[:, 0:64], wm_a[:, 0:ap],
                start=True, stop=True,
            )

        for _ in range(CFG["warmup"]):
            junk_mm(CFG["warmup_ap"])

        # all bases' sketch-norm sums accumulate into one [16, BN] psum bank
        # (row 2j = sum zq^2 of basis j, row 2j+1 = sum zk^2)
        n2_all = n2_ps.tile([2 * K, BN], F32, tag="n2")
        tail_sb = tail_pool.tile([128, 2 * BN], BF16)

        # --- fused = concat @ fw.T (+ fb), transposed, fp8 -----------------
        fusedT = fused_pool.tile([128, DC * BN], FP8)
        fusedT_v = fusedT.rearrange("p (d w) -> p d w", w=BN)
        PB = 2 * D + 2 * BN  # one interleaved pair-block: fw pair + concat pair
        with tc.tile_pool(name="prep", bufs=1) as prep_pool:
            big = prep_pool.tile([128, DC * PB], FP8)
            # three chunks: [pairs 0,1] then pair 2 and pair 3 separately,
            # so the c=2 round starts on its own completion sem
            for lo, hi in ((0, 2), (2, 3), (3, 4)):
                nc.sync.dma_start(
                    big[:, lo * PB : hi * PB],
                    big_in[:, lo * PB : hi * PB],
                )
                if lo == 0:
                    nc.sync.dma_start(fb_sb[:], fb_in)

            def fw_pair(c):
                return big[:, c * PB : c * PB + 2 * D].rearrange(
                    "p (c f) -> p c f", f=D
                )

            def conc_pair(c):
                return big[:, c * PB + 2 * D : (c + 1) * PB].rearrange(
                    "p (c w) -> p c w", w=BN
                )

            fps_t = [
                uv_ps.tile([128, BN], F32, tag="uv", name=f"fps{f}")
                for f in range(DC)
            ]
            fps = [t[:] for t in fps_t]
            # the bias rank-1 updates OPEN each accumulation group: they only
            # need fb/ones, so they run during the junk window before the
            # first concat chunk lands
            for f in range(DC):
                nc.tensor.matmul(
                    fps[f],
                    fb_sb[:, f * 128 : (f + 1) * 128],
                    ones_row[:],
                    start=True,
                    stop=False,
                )
            # first two c-rounds c-major (chunk A), last two f-major (chunk
            # B) so each f-bank's stop comes in copy order and the f0/f1
            # copies (which gate the first uv-proj) start 8 matmuls earlier
            for c in range(2):
                for f in range(DC):
                    nc.tensor.matmul(
                        fps[f],
                        fw_pair(c)[:, :, f * 128 : (f + 1) * 128],
                        conc_pair(c)[:],
                        start=False,
                        stop=False,
                        perf_mode=DR,
                    )
            for _ in range(CFG["warmup_mid"]):
                junk_mm(256)
            for f in range(DC):
                for c in range(2, DC):
                    nc.tensor.matmul(
                        fps[f],
                        fw_pair(c)[:, :, f * 128 : (f + 1) * 128],
                        conc_pair(c)[:],
                        start=False,
                        stop=(c == DC - 1),
                        perf_mode=DR,
                    )
            for f in range(DC):
                copy_psum(CFG["fused"][f % 2], fusedT_v[:, f, :], fps[f])

        # --- per-basis pipeline --------------------------------------------
        w_sbs = {}

        def load_w(j):
            w_sb = w_pool.tile([128, DC * WCOLS], FP8, tag="w")
            nc.sync.dma_start(w_sb[:], wam[j])
            w_sbs[j] = w_sb.rearrange("p (c w) -> p c w", w=WCOLS)

        def uvproj(j, u_sb, v_sb):
            # u = fused @ U128, v = fused @ (VS)128: separate single-bank
            # psum tiles and separate SBUF tiles so the two copies share no
            # tile (same-tile writers serialize in the scheduler)
            w_v = w_sbs[j]
            engs = CFG["uv_copy_last"] if j == K - 1 else CFG["uv_copy"][j % 2]
            for side, dst in ((0, u_sb), (1, v_sb)):
                ps = uv_ps.tile([128, BN], F32, tag="uv")
                for i in range(2):
                    nc.tensor.matmul(
                        ps[:],
                        w_v[:, 2 * i : 2 * i + 2,
                            side * RA : (side + 1) * RA],
                        fusedT_v[:, 2 * i : 2 * i + 2, :],
                        start=(i == 0),
                        stop=(i == 1),
                        perf_mode=DR,
                    )
                copy_psum(engs[side], dst[:], ps[:])

        def zproj(j, pool=None, tag="z"):
            # z = fused @ [Mq | Mk]: one [128, BN] psum bank (64 zq + 64 zk)
            w_v = w_sbs[j]
            zp = (pool or z_ps).tile([128, BN], F32, tag=tag)
            for i in range(2):
                nc.tensor.matmul(
                    zp[:],
                    w_v[:, 2 * i : 2 * i + 2, 2 * RA : 2 * RA + 2 * R],
                    fusedT_v[:, 2 * i : 2 * i + 2, :],
                    start=(i == 0),
                    stop=(i == 1),
                    perf_mode=DR,
                )
            return zp

        def square(j, zp, sq_v):
            # sq = (z/16)^2 straight from psum into fp8 (the 1/16 is folded
            # into M on the host); ACT Square is the only psum-legal square
            nc.scalar.activation(sq_v[:, j % 2, :], zp[:], AF.Square)

        def ones_mm(pair, sq_tile):
            # one DR matmul reduces both bases' (q,k) sketch squares over
            # partitions into rows 4p..4p+3 of n2_all
            nc.tensor.matmul(
                n2_all[:],
                oh_v[:, :, 12 - 4 * pair : 28 - 4 * pair],
                sq_tile[:],
                start=(pair == 0),
                stop=(pair == K // 2 - 1),
                perf_mode=DR,
                skip_group_check=True,
            )

        def logits(j, u_sb, v_sb):
            # lg[n, m] = sum_r u[r, n] v[r, m]: single-chunk contraction, one
            # matmul per local batch
            lg = lg_ps.tile([128, BN], F32, tag="lg")
            for b in range(BL):
                bs = slice(b * 128, (b + 1) * 128)
                nc.tensor.matmul(
                    lg[:, bs], u_sb[:, bs], v_sb[:, bs],
                    start=True, stop=True,
                )
            if j == K - 1:
                copy_psum(CFG["lg_last"], tail_sb[:, 0:BN], lg[:])
                nc.sync.dma_start(tail_out, tail_sb[:])
                return
            lgs = lg_sb_pool.tile([128, BN], BF16, tag="lgs")
            copy_psum(CFG["lg"][j], lgs[:], lg[:])
            nc.sync.dma_start(lg_out[j], lgs[:])

        load_w(0)
        load_w(1)
        prev = None
        sq_tile = None
        for j in range(K):
            if j + 2 < K:
                load_w(j + 2)
            u_sb = u_pool.tile([128, BN], FP8, tag="u")
            v_sb = v_pool.tile([128, BN], FP8, tag="v")

            def z_work(j=j, pool=None, tag="z"):
                nonlocal sq_tile
                zp = zproj(j, pool, tag)
                if j % 2 == 0:
                    sq_tile = sq_pool.tile([128, 2 * BN], FP8, tag="sq")
                sq_v = sq_tile.rearrange("p (d w) -> p d w", w=BN)
                square(j, zp, sq_v)
                if j % 2 == 1:
                    ones_mm(j // 2, sq_v)

            uvproj(j, u_sb, v_sb)
            if prev is not None:
                logits(prev[2], prev[0], prev[1])
            if j < K - 1:
                z_work()
            if j == K - 2:
                # the last basis' sketch work runs a cycle early (w is
                # already loaded) so nothing norm-related sits in the tail
                with tc.high_priority(offset=CFG["z_prio"]):
                    # the last basis' z borrows an lg psum tile: no wait on
                    # the z bank (still busy with sq6), and logits(7)
                    # reclaims it only after sq7 - which is early now
                    z_work(K - 1, lg_ps, "lg")
                    copy_psum(CFG["n2"], tail_sb[0 : 2 * K, BN:], n2_all[:])
            prev = (u_sb, v_sb, j)
            del w_sbs[j]

        logits(prev[2], prev[0], prev[1])


_CACHE = {}


def _get_nc():
    if "nc" not in _CACHE:
        _CACHE["nc"] = build_kernel()
    return _CACHE["nc"]


def shard_inputs(desc_embeddings, name_value_embeddings, W_q, W_k, fusion_w, fusion_b):
    import ml_dtypes

    fp8 = ml_dtypes.float8_e4m3
    big = np.concatenate(
        [np.asarray(desc_embeddings, np.float32), np.asarray(name_value_embeddings, np.float32)],
        axis=-1,
    )  # [B, N, 2D]
    fwt = (
        np.asarray(fusion_w, np.float32).T.reshape(CC, 128, D)
        .transpose(1, 0, 2).reshape(128, CC * D).astype(fp8)
    )
    fb_row = np.ascontiguousarray(
        np.asarray(fusion_b, np.float32)[None, :].astype(ml_dtypes.bfloat16)
    )

    W_q = np.asarray(W_q, np.float32)
    W_k = np.asarray(W_k, np.float32)
    Pq = np.random.default_rng(1).standard_normal((R, D)).astype(np.float32)
    Pk = np.random.default_rng(2).standard_normal((R, D)).astype(np.float32)
    wam = np.empty((K, 128, DC * WCOLS), dtype=fp8)
    for k in range(K):
        A = W_q[k] @ W_k[k].T
        U, S, Vt = np.linalg.svd(A)
        Ur = U[:, :RA] * U_SCALE                             # [D, RA]
        Vr = Vt[:RA].T * S[:RA] * V_SCALE                    # [D, RA]
        Mq = (W_q[k] @ Pq.T) * Z_SCALE                       # [D, R]
        Mk = (W_k[k] @ Pk.T) * Z_SCALE
        blk = np.concatenate([Ur, Vr, Mq, Mk], axis=1)       # [D, WCOLS]
        wam[k] = blk.reshape(DC, 128, WCOLS).transpose(1, 0, 2).reshape(
            128, DC * WCOLS).astype(fp8)

    full = {"fb_in": fb_row, "wam": wam}
    in_maps = []
    for c in range(CORES):
        x = big[c * BL : (c + 1) * BL]  # [BL, N, 2D]
        img = (
            x.transpose(2, 0, 1).reshape(CC, 128, BL * N)
            .transpose(1, 0, 2).reshape(128, CC * BN).astype(fp8)
        )
        blocks = []
        for cc in range(DC):
            blocks.append(fwt[:, cc * 2 * D : (cc + 1) * 2 * D])
            blocks.append(img[:, cc * 2 * BN : (cc + 1) * 2 * BN])
        m = dict(full)
        m["big_in"] = np.ascontiguousarray(np.concatenate(blocks, axis=1))
        in_maps.append(m)
    return in_maps


def assemble_outputs(results):
    tail = np.stack([np.asarray(r["tail_out"], np.float32) for r in results])
    lg = np.stack([np.asarray(r["lg_out"], np.float32) for r in results])
    lg = np.concatenate([lg, tail[:, None, :, 0:BN]], axis=1)  # [C, K, N, BN]
    n2 = tail[:, 0 : 2 * K, BN:]                               # [C, 2K, BN]
    # [C, K, N, BL*N] -> [B, K, N, N]
    lg = lg.reshape(CORES, K, N, BL, N).transpose(0, 3, 1, 2, 4).reshape(B, K, N, N)
    # n2 rows 2j/2j+1 = sum (z/16)^2 = |z|^2/256; n2_hat = |z|^2/R
    n2 = n2.reshape(CORES, K, 2, BL, N) * (1.0 / (Z_SCALE * Z_SCALE * R))
    inv = 1.0 / np.sqrt(np.maximum(n2, 1e-24))              # [C, K, 2, BL, N]
    invq = inv[:, :, 0].transpose(0, 2, 1, 3).reshape(B, K, N)
    invk = inv[:, :, 1].transpose(0, 2, 1, 3).reshape(B, K, N)
    logits = lg * invq[:, :, :, None] * invk[:, :, None, :] * (
        1.0 / (U_SCALE * V_SCALE * np.sqrt(D))
    )
    mx = logits.max(-1, keepdims=True)
    e = np.exp(logits - mx)
    alpha = (e / e.sum(-1, keepdims=True)).astype(np.float32)
    ema = np.float32(1.0 - MOMENTUM) * alpha.mean(0)
    bias_log = np.log(np.maximum(ema, np.float32(EPS)))
    bias_log = np.broadcast_to(bias_log[None], (B, K, N, N)).astype(np.float32)
    return bias_log, alpha


def kernel(desc_embeddings, name_value_embeddings, W_q, W_k, fusion_w, fusion_b,
           _trace=False):
    nc = _get_nc()
    in_maps = shard_inputs(
        desc_embeddings, name_value_embeddings, W_q, W_k, fusion_w, fusion_b
    )
    res = run_bass_kernel_spmd(nc, in_maps, core_ids=list(range(CORES)), trace=_trace)
    out = assemble_outputs(res.results)
    if _trace:
        return out, res
    return out


# revision 71
# speedup vs baseline: 1.8427x; 1.0138x over previous
"""Trainium2 Bass kernel for BasisAffinityGAT (8-core data-parallel over batch).

Per batch b:
  fused = concat(desc, nv) @ fusion_w.T + fusion_b          [N, D]
  q_k = l2norm(fused @ W_q[k]); k_k = l2norm(fused @ W_k[k])
  alpha[b,k] = softmax(q_k @ k_k.T / sqrt(D))               [K, N, N]
Outputs: (bias_log, alpha), bias_log = log(max(0.01*mean_b(alpha), 1e-6)).

Device strategy (v4): batch sharded 4-per-core; per-basis weights replaced by
host-precomputed factors. The raw affinity q_k k_k^T == fused A_k fused^T with
A_k = W_q[k] W_k[k]^T; A_k is truncated to its top-128 singular directions
(A ~ U S V^T, U128 and V128 S128 shipped as fp8) - the softmax's tiny logit
scale makes the truncation error negligible (verified ~6e-3 rel on alpha).
The device runs two half-size fp8 DoubleRow projections u = fused @ U128 and
v = fused @ (V S)128 per basis, then single-chunk fp8 logits lg = u v^T.
The l2 norms come from r=64 JL sketches: z = fused @ M_k with
M_k = [W_q P_q^T | W_k P_k^T]/16 (64+64 partitions in one psum bank), squared
via ACT Square straight from psum into fp8 (the only psum-legal square: pool
cannot read PSUM and DVE cannot read one psum AP twice), and reduced over
partitions by one DoubleRow ones-matmul per basis pair into a single [16, BN]
psum accumulator. Raw logits (bf16) leave per basis; the last basis' logits
and the n2 rows ride in one tail tensor (single HWDGE+sem chain), with its
sketch work hoisted a cycle early (z borrowing an lg psum bank). The host
finishes the scalar math: inverse norms, scales, softmax, batch-mean EMA and
bias_log. u and v use separate SBUF tiles (same-tile writers serialize in
the tile scheduler) and the lgs pool is 4-deep to break the WAR chain
through DMA completions. fusion_b enters as rank-1 ones-matmuls that OPEN
the fused psum accumulation groups (they run during the DMA prologue); junk
warmup matmuls hold the PE clock at full p-state through the prologue.
"""

import os
import sys

import numpy as np

# The kernel executes through jax's axon PJRT backend; a JAX_PLATFORMS=cpu
# pin (common for running the jax reference) would hide the NeuronCores.
if "axon" not in os.environ.get("JAX_PLATFORMS", "axon"):
    os.environ.pop("JAX_PLATFORMS", None)

try:
    import concourse  # noqa: F401
except ImportError:  # pragma: no cover
    sys.path.insert(0, "/opt/trn_rl_repo")

import concourse.tile as tile  # noqa: E402
from concourse import bacc, mybir  # noqa: E402
from concourse.bass_utils import run_bass_kernel_spmd  # noqa: E402

B, N, D, K = 32, 128, 512, 8
CORES = 8
BL = B // CORES          # local batch per core
DC = D // 128            # 4 chunks of the feature dim
CC = 2 * D // 128        # 8 chunks of the concat dim
MOMENTUM = 0.99
EPS = 1e-6
R = 64                   # JL sketch size per side
RA = 128                 # rank kept of A = Wq Wk^T
U_SCALE = 16.0           # fp8 dynamic-range scale on U (orthonormal cols)
V_SCALE = 8.0            # fp8 scale on V*S
Z_SCALE = 1.0 / 16.0     # folded into M on the host; undone in assemble

F32 = mybir.dt.float32
BF16 = mybir.dt.bfloat16
FP8 = mybir.dt.float8e4
AF = mybir.ActivationFunctionType
DR = mybir.MatmulPerfMode.DoubleRow

BN = BL * N              # 512: free dim packing all local batches
WCOLS = RA + RA + 2 * R  # per-dchunk stationary block: U | VS | Mq Mk


def build_kernel():
    nc = bacc.Bacc(
        "TRN2",
        target_bir_lowering=False,
        debug=False,
        enable_asserts=False,
    )

    # fwT (interleaved with concatT per contraction pair) as one image so the
    # fused matmuls chase the input DMA
    big_in = nc.dram_tensor("big_in", [128, CC * D + CC * BN], FP8, kind="ExternalInput").ap()
    fb_in = nc.dram_tensor("fb_in", [1, D], BF16, kind="ExternalInput").ap()
    # per basis: [4 dchunk, 128 U-cols + 128 VS-cols + 128 M-cols]
    wam = nc.dram_tensor("wam", [K, 128, DC * WCOLS], FP8, kind="ExternalInput").ap()
    lg_out = nc.dram_tensor("lg_out", [K - 1, N, BN], BF16, kind="ExternalOutput").ap()
    # last basis logits (cols 0:BN) + the n2 rows (cols BN:, rows 0:2K)
    tail_out = nc.dram_tensor("tail_out", [128, BN + BN], BF16, kind="ExternalOutput").ap()

    with tile.TileContext(nc) as tc:
        _emit(tc, big_in, fb_in, wam, lg_out, tail_out)
    nc.finalize()
    return nc


# Engine-assignment knobs (tuned against TimelineSim). pool (gpsimd) cannot
# read PSUM on real hw (BIR verifier rejects it) and DVE cannot read the same
# psum AP twice, so psum copies live on ACT/DVE and the sketch square always
# uses ACT's Square activation.
CFG = {
    "uv_copy": [("act", "dve"), ("act", "dve")],
    "uv_copy_last": ("act", "dve"),
    "lg": ["dve", "dve", "dve", "dve", "dve", "dve", "dve", "act"],
    "lg_last": "act",
    "n2": "dve",
    "fused": ("dve", "act"),
    "warmup": 9,
    "warmup_ap": 256,
    "warmup_mid": 5,
    "z_prio": 60,
}


def _emit(tc, big_in, fb_in, wam, lg_out, tail_out):
    nc = tc.nc
    from contextlib import ExitStack

    def copy_psum(engine, dst, src):
        if engine == "dve":
            nc.vector.tensor_copy(dst, src)
        else:
            nc.scalar.activation(dst, src, AF.Copy)

    ctx = ExitStack()
    with ctx:
        const_pool = ctx.enter_context(tc.tile_pool(name="const", bufs=1))
        fused_pool = ctx.enter_context(tc.tile_pool(name="fused", bufs=1))
        w_pool = ctx.enter_context(tc.tile_pool(name="w", bufs=3))
        u_pool = ctx.enter_context(tc.tile_pool(name="u", bufs=2))
        v_pool = ctx.enter_context(tc.tile_pool(name="v", bufs=2))
        sq_pool = ctx.enter_context(tc.tile_pool(name="sq", bufs=2))
        lg_sb_pool = ctx.enter_context(tc.tile_pool(name="lgsb", bufs=4))
        tail_pool = ctx.enter_context(tc.tile_pool(name="tail", bufs=1))
        # 4 single-bank uv tiles + 1 z bank + 2 lg banks + 1 n2 bank = 8
        uv_ps = ctx.enter_context(tc.tile_pool(name="uv_ps", bufs=4, space="PSUM"))
        z_ps = ctx.enter_context(tc.tile_pool(name="z_ps", bufs=1, space="PSUM"))
        n2_ps = ctx.enter_context(tc.tile_pool(name="n2_ps", bufs=1, space="PSUM"))
        lg_ps = ctx.enter_context(tc.tile_pool(name="lg_ps", bufs=2, space="PSUM"))

        # PE warmup: junk matmuls hold the tensor engine busy through the
        # input-DMA prologue so the p-state ramp lands on junk. The operand
        # is uninitialized SBUF garbage (never read downstream; the warmup
        # psum rows are fully overwritten by the first start=True logits).
        # The one-column memset (allocates the tile) must be DVE's FIRST op.
        wmp = lg_ps.tile([128, BN], F32, tag="lg")
        wm_a = const_pool.tile([128, 256], FP8)
        nc.vector.memset(wm_a[:, 0:1], 0.0)

        # one-hot selector image for the norm-reduce matmuls: sliced
        # [128, 2, 16] windows all see ones at in-window col 4p + 2c +
        # (partition>=64) regardless of the pair index p.
        # chunk stride must be a multiple of 16B for dual-fp8 Ldweights
        oh = const_pool.tile([128, 2 * 32], FP8)
        oh_v = oh.rearrange("p (c w) -> p c w", w=32)
        nc.gpsimd.memset(oh[:], 0.0)
        nc.gpsimd.memset(oh_v[0:64, 0, 12:13], 1.0)
        nc.gpsimd.memset(oh_v[64:128, 0, 13:14], 1.0)
        nc.gpsimd.memset(oh_v[0:64, 1, 14:15], 1.0)
        nc.gpsimd.memset(oh_v[64:128, 1, 15:16], 1.0)

        fb_sb = const_pool.tile([1, D], BF16)
        ones_row = const_pool.tile([1, BN], BF16)
        nc.vector.memset(ones_row[:], 1.0)

        def junk_mm(ap):
            nc.tensor.matmul(
                wmp[0:64, 0:ap], wm_a[:, 0:64], wm_a[:, 0:ap],
                start=True, stop=True,
            )

        for _ in range(CFG["warmup"]):
            junk_mm(CFG["warmup_ap"])

        # all bases' sketch-norm sums accumulate into one [16, BN] psum bank
        # (row 2j = sum zq^2 of basis j, row 2j+1 = sum zk^2)
        n2_all = n2_ps.tile([2 * K, BN], F32, tag="n2")
        tail_sb = tail_pool.tile([128, 2 * BN], BF16)

        # --- fused = concat @ fw.T (+ fb), transposed, fp8 -----------------
        fusedT = fused_pool.tile([128, DC * BN], FP8)
        fusedT_v = fusedT.rearrange("p (d w) -> p d w", w=BN)
        PB = 2 * D + 2 * BN  # one interleaved pair-block: fw pair + concat pair
        with tc.tile_pool(name="prep", bufs=1) as prep_pool:
            big = prep_pool.tile([128, DC * PB], FP8)
            # three chunks: [pairs 0,1] then pair 2 and pair 3 separately,
            # so the c=2 round starts on its own completion sem
            for lo, hi in ((0, 2), (2, 3), (3, 4)):
                nc.sync.dma_start(
                    big[:, lo * PB : hi * PB],
                    big_in[:, lo * PB : hi * PB],
                )
                if lo == 0:
                    nc.sync.dma_start(fb_sb[:], fb_in)

            def fw_pair(c):
                return big[:, c * PB : c * PB + 2 * D].rearrange(
                    "p (c f) -> p c f", f=D
                )

            def conc_pair(c):
                return big[:, c * PB + 2 * D : (c + 1) * PB].rearrange(
                    "p (c w) -> p c w", w=BN
                )

            fps_t = [
                uv_ps.tile([128, BN], F32, tag="uv", name=f"fps{f}")
                for f in range(DC)
            ]
            fps = [t[:] for t in fps_t]
            # the bias rank-1 updates OPEN each accumulation group: they only
            # need fb/ones, so they run during the junk window before the
            # first concat chunk lands
            for f in range(DC):
                nc.tensor.matmul(
                    fps[f],
                    fb_sb[:, f * 128 : (f + 1) * 128],
                    ones_row[:],
                    start=True,
                    stop=False,
                )
            # first two c-rounds c-major (chunk A), last two f-major (chunk
            # B) so each f-bank's stop comes in copy order and the f0/f1
            # copies (which gate the first uv-proj) start 8 matmuls earlier
            for c in range(2):
                for f in range(DC):
                    nc.tensor.matmul(
                        fps[f],
                        fw_pair(c)[:, :, f * 128 : (f + 1) * 128],
                        conc_pair(c)[:],
                        start=False,
                        stop=False,
                        perf_mode=DR,
                    )
            for _ in range(CFG["warmup_mid"]):
                junk_mm(256)
            for f in range(DC):
                for c in range(2, DC):
                    nc.tensor.matmul(
                        fps[f],
                        fw_pair(c)[:, :, f * 128 : (f + 1) * 128],
                        conc_pair(c)[:],
                        start=False,
                        stop=(c == DC - 1),
                        perf_mode=DR,
                    )
            for f in range(DC):
                copy_psum(CFG["fused"][f % 2], fusedT_v[:, f, :], fps[f])

        # --- per-basis pipeline --------------------------------------------
        w_sbs = {}

        def load_w(j):
            w_sb = w_pool.tile([128, DC * WCOLS], FP8, tag="w")
            nc.sync.dma_start(w_sb[:], wam[j])
            w_sbs[j] = w_sb.rearrange("p (c w) -> p c w", w=WCOLS)

        def uvproj(j, u_sb, v_sb):
            # u = fused @ U128, v = fused @ (VS)128: separate single-bank
            # psum tiles and separate SBUF tiles so the two copies share no
            # tile (same-tile writers serialize in the scheduler)
            w_v = w_sbs[j]
            engs = CFG["uv_copy_last"] if j == K - 1 else CFG["uv_copy"][j % 2]
            for side, dst in ((0, u_sb), (1, v_sb)):
                ps = uv_ps.tile([128, BN], F32, tag="uv")
                for i in range(2):
                    nc.tensor.matmul(
                        ps[:],
                        w_v[:, 2 * i : 2 * i + 2,
                            side * RA : (side + 1) * RA],
                        fusedT_v[:, 2 * i : 2 * i + 2, :],
                        start=(i == 0),
                        stop=(i == 1),
                        perf_mode=DR,
                    )
                copy_psum(engs[side], dst[:], ps[:])

        def zproj(j, pool=None, tag="z"):
            # z = fused @ [Mq | Mk]: one [128, BN] psum bank (64 zq + 64 zk)
            w_v = w_sbs[j]
            zp = (pool or z_ps).tile([128, BN], F32, tag=tag)
            for i in range(2):
                nc.tensor.matmul(
                    zp[:],
                    w_v[:, 2 * i : 2 * i + 2, 2 * RA : 2 * RA + 2 * R],
                    fusedT_v[:, 2 * i : 2 * i + 2, :],
                    start=(i == 0),
                    stop=(i == 1),
                    perf_mode=DR,
                )
            return zp

        def square(j, zp, sq_v):
            # sq = (z/16)^2 straight from psum into fp8 (the 1/16 is folded
            # into M on the host); ACT Square is the only psum-legal square
            nc.scalar.activation(sq_v[:, j % 2, :], zp[:], AF.Square)

        def ones_mm(pair, sq_tile):
            # one DR matmul reduces both bases' (q,k) sketch squares over
            # partitions into rows 4p..4p+3 of n2_all
            nc.tensor.matmul(
                n2_all[:],
                oh_v[:, :, 12 - 4 * pair : 28 - 4 * pair],
                sq_tile[:],
                start=(pair == 0),
                stop=(pair == K // 2 - 1),
                perf_mode=DR,
                skip_group_check=True,
            )

        def logits(j, u_sb, v_sb):
            # lg[n, m] = sum_r u[r, n] v[r, m]: single-chunk contraction, one
            # matmul per local batch
            lg = lg_ps.tile([128, BN], F32, tag="lg")
            for b in range(BL):
                bs = slice(b * 128, (b + 1) * 128)
                nc.tensor.matmul(
                    lg[:, bs], u_sb[:, bs], v_sb[:, bs],
                    start=True, stop=True,
                )
            if j == K - 1:
                copy_psum(CFG["lg_last"], tail_sb[:, 0:BN], lg[:])
                nc.sync.dma_start(tail_out, tail_sb[:])
                return
            lgs = lg_sb_pool.tile([128, BN], BF16, tag="lgs")
            copy_psum(CFG["lg"][j], lgs[:], lg[:])
            nc.sync.dma_start(lg_out[j], lgs[:])

        load_w(0)
        load_w(1)
        prev = None
        sq_tile = None
        for j in range(K):
            if j + 2 < K:
                load_w(j + 2)
            u_sb = u_pool.tile([128, BN], FP8, tag="u")
            v_sb = v_pool.tile([128, BN], FP8, tag="v")

            def z_work(j=j, pool=None, tag="z"):
                nonlocal sq_tile
                zp = zproj(j, pool, tag)
                if j % 2 == 0:
                    sq_tile = sq_pool.tile([128, 2 * BN], FP8, tag="sq")
                sq_v = sq_tile.rearrange("p (d w) -> p d w", w=BN)
                square(j, zp, sq_v)
                if j % 2 == 1:
                    ones_mm(j // 2, sq_v)

            uvproj(j, u_sb, v_sb)
            if prev is not None:
                logits(prev[2], prev[0], prev[1])
            if j < K - 1:
                z_work()
            if j == K - 2:
                # the last basis' sketch work runs a cycle early (w is
                # already loaded) so nothing norm-related sits in the tail
                with tc.high_priority(offset=CFG["z_prio"]):
                    # the last basis' z borrows an lg psum tile: no wait on
                    # the z bank (still busy with sq6), and logits(7)
                    # reclaims it only after sq7 - which is early now
                    z_work(K - 1, lg_ps, "lg")
                    copy_psum(CFG["n2"], tail_sb[0 : 2 * K, BN:], n2_all[:])
            prev = (u_sb, v_sb, j)
            del w_sbs[j]

        logits(prev[2], prev[0], prev[1])


_CACHE = {}


def _get_nc():
    if "nc" not in _CACHE:
        _CACHE["nc"] = build_kernel()
    return _CACHE["nc"]


def shard_inputs(desc_embeddings, name_value_embeddings, W_q, W_k, fusion_w, fusion_b):
    import ml_dtypes

    fp8 = ml_dtypes.float8_e4m3
    big = np.concatenate(
        [np.asarray(desc_embeddings, np.float32), np.asarray(name_value_embeddings, np.float32)],
        axis=-1,
    )  # [B, N, 2D]
    fwt = (
        np.asarray(fusion_w, np.float32).T.reshape(CC, 128, D)
        .transpose(1, 0, 2).reshape(128, CC * D).astype(fp8)
    )
    fb_row = np.ascontiguousarray(
        np.asarray(fusion_b, np.float32)[None, :].astype(ml_dtypes.bfloat16)
    )

    W_q = np.asarray(W_q, np.float32)
    W_k = np.asarray(W_k, np.float32)
    Pq = np.random.default_rng(1).standard_normal((R, D)).astype(np.float32)
    Pk = np.random.default_rng(2).standard_normal((R, D)).astype(np.float32)
    wam = np.empty((K, 128, DC * WCOLS), dtype=fp8)
    for k in range(K):
        A = W_q[k] @ W_k[k].T
        U, S, Vt = np.linalg.svd(A)
        Ur = U[:, :RA] * U_SCALE                             # [D, RA]
        Vr = Vt[:RA].T * S[:RA] * V_SCALE                    # [D, RA]
        Mq = (W_q[k] @ Pq.T) * Z_SCALE                       # [D, R]
        Mk = (W_k[k] @ Pk.T) * Z_SCALE
        blk = np.concatenate([Ur, Vr, Mq, Mk], axis=1)       # [D, WCOLS]
        wam[k] = blk.reshape(DC, 128, WCOLS).transpose(1, 0, 2).reshape(
            128, DC * WCOLS).astype(fp8)

    full = {"fb_in": fb_row, "wam": wam}
    in_maps = []
    for c in range(CORES):
        x = big[c * BL : (c + 1) * BL]  # [BL, N, 2D]
        img = (
            x.transpose(2, 0, 1).reshape(CC, 128, BL * N)
            .transpose(1, 0, 2).reshape(128, CC * BN).astype(fp8)
        )
        blocks = []
        for cc in range(DC):
            blocks.append(fwt[:, cc * 2 * D : (cc + 1) * 2 * D])
            blocks.append(img[:, cc * 2 * BN : (cc + 1) * 2 * BN])
        m = dict(full)
        m["big_in"] = np.ascontiguousarray(np.concatenate(blocks, axis=1))
        in_maps.append(m)
    return in_maps


def assemble_outputs(results):
    tail = np.stack([np.asarray(r["tail_out"], np.float32) for r in results])
    lg = np.stack([np.asarray(r["lg_out"], np.float32) for r in results])
    lg = np.concatenate([lg, tail[:, None, :, 0:BN]], axis=1)  # [C, K, N, BN]
    n2 = tail[:, 0 : 2 * K, BN:]                               # [C, 2K, BN]
    # [C, K, N, BL*N] -> [B, K, N, N]
    lg = lg.reshape(CORES, K, N, BL, N).transpose(0, 3, 1, 2, 4).reshape(B, K, N, N)
    # n2 rows 2j/2j+1 = sum (z/16)^2 = |z|^2/256; n2_hat = |z|^2/R
    n2 = n2.reshape(CORES, K, 2, BL, N) * (1.0 / (Z_SCALE * Z_SCALE * R))
    inv = 1.0 / np.sqrt(np.maximum(n2, 1e-24))              # [C, K, 2, BL, N]
    invq = inv[:, :, 0].transpose(0, 2, 1, 3).reshape(B, K, N)
    invk = inv[:, :, 1].transpose(0, 2, 1, 3).reshape(B, K, N)
    logits = lg * invq[:, :, :, None] * invk[:, :, None, :] * (
        1.0 / (U_SCALE * V_SCALE * np.sqrt(D))
    )
    mx = logits.max(-1, keepdims=True)
    e = np.exp(logits - mx)
    alpha = (e / e.sum(-1, keepdims=True)).astype(np.float32)
    ema = np.float32(1.0 - MOMENTUM) * alpha.mean(0)
    bias_log = np.log(np.maximum(ema, np.float32(EPS)))
    bias_log = np.broadcast_to(bias_log[None], (B, K, N, N)).astype(np.float32)
    return bias_log, alpha


def kernel(desc_embeddings, name_value_embeddings, W_q, W_k, fusion_w, fusion_b,
           _trace=False):
    nc = _get_nc()
    in_maps = shard_inputs(
        desc_embeddings, name_value_embeddings, W_q, W_k, fusion_w, fusion_b
    )
    res = run_bass_kernel_spmd(nc, in_maps, core_ids=list(range(CORES)), trace=_trace)
    out = assemble_outputs(res.results)
    if _trace:
        return out, res
    return out


# revision 77
# speedup vs baseline: 1.8810x; 1.0208x over previous
"""Trainium2 Bass kernel for BasisAffinityGAT (8-core data-parallel over batch).

Per batch b:
  fused = concat(desc, nv) @ fusion_w.T + fusion_b          [N, D]
  q_k = l2norm(fused @ W_q[k]); k_k = l2norm(fused @ W_k[k])
  alpha[b,k] = softmax(q_k @ k_k.T / sqrt(D))               [K, N, N]
Outputs: (bias_log, alpha), bias_log = log(max(0.01*mean_b(alpha), 1e-6)).

Device strategy (v4): batch sharded 4-per-core; per-basis weights replaced by
host-precomputed factors. The raw affinity q_k k_k^T == fused A_k fused^T with
A_k = W_q[k] W_k[k]^T; A_k is truncated to its top-128 singular directions
(A ~ U S V^T, U128 and V128 S128 shipped as fp8) - the softmax's tiny logit
scale makes the truncation error negligible (verified ~6e-3 rel on alpha).
The device runs two half-size fp8 DoubleRow projections u = fused @ U128 and
v = fused @ (V S)128 per basis, then single-chunk fp8 logits lg = u v^T.
The l2 norms come from r=64 JL sketches: z = fused @ M_k with
M_k = [W_q P_q^T | W_k P_k^T]/16 (64+64 partitions in one psum bank), squared
via ACT Square straight from psum into fp8 (the only psum-legal square: pool
cannot read PSUM and DVE cannot read one psum AP twice), and reduced over
partitions by one DoubleRow ones-matmul per basis pair into a single [16, BN]
psum accumulator. Raw logits (bf16) leave per basis; the last basis' logits
and the n2 rows ride in one tail tensor (single HWDGE+sem chain), with its
sketch work hoisted a cycle early (z borrowing an lg psum bank). The host
finishes the scalar math: inverse norms, scales, softmax, batch-mean EMA and
bias_log. u and v use separate SBUF tiles (same-tile writers serialize in
the tile scheduler) and the lgs pool is 4-deep to break the WAR chain
through DMA completions. fusion_b enters as rank-1 ones-matmuls that OPEN
the fused psum accumulation groups (they run during the DMA prologue); junk
warmup matmuls hold the PE clock at full p-state through the prologue.
"""

import os
import sys

import numpy as np

# The kernel executes through jax's axon PJRT backend; a JAX_PLATFORMS=cpu
# pin (common for running the jax reference) would hide the NeuronCores.
if "axon" not in os.environ.get("JAX_PLATFORMS", "axon"):
    os.environ.pop("JAX_PLATFORMS", None)

try:
    import concourse  # noqa: F401
except ImportError:  # pragma: no cover
    sys.path.insert(0, "/opt/trn_rl_repo")

import concourse.tile as tile  # noqa: E402
from concourse import bacc, mybir  # noqa: E402
from concourse.bass_utils import run_bass_kernel_spmd  # noqa: E402

B, N, D, K = 32, 128, 512, 8
CORES = 8
BL = B // CORES          # local batch per core
DC = D // 128            # 4 chunks of the feature dim
CC = 2 * D // 128        # 8 chunks of the concat dim
MOMENTUM = 0.99
EPS = 1e-6
R = 32                   # JL sketch size per side (pair-packed)
RA = 128                 # rank kept of A = Wq Wk^T
U_SCALE = 16.0           # fp8 dynamic-range scale on U (orthonormal cols)
V_SCALE = 8.0            # fp8 scale on V*S
Z_SCALE = 1.0 / 16.0     # folded into M on the host; undone in assemble

F32 = mybir.dt.float32
BF16 = mybir.dt.bfloat16
FP8 = mybir.dt.float8e4
AF = mybir.ActivationFunctionType
DR = mybir.MatmulPerfMode.DoubleRow

BN = BL * N              # 512: free dim packing all local batches
WCOLS = RA + RA + 128    # per-dchunk stationary block: U | VS | padded M


def build_kernel():
    nc = bacc.Bacc(
        "TRN2",
        target_bir_lowering=False,
        debug=False,
        enable_asserts=False,
    )

    # fwT (interleaved with concatT per contraction pair) as one image so the
    # fused matmuls chase the input DMA
    big_in = nc.dram_tensor("big_in", [128, CC * D + CC * BN], FP8, kind="ExternalInput").ap()
    fb_in = nc.dram_tensor("fb_in", [1, D], BF16, kind="ExternalInput").ap()
    # per basis: [4 dchunk, 128 U-cols + 128 VS-cols + 128 M-cols]
    wam = nc.dram_tensor("wam", [K, 128, DC * WCOLS], FP8, kind="ExternalInput").ap()
    lg_out = nc.dram_tensor("lg_out", [K - 1, N, BN], BF16, kind="ExternalOutput").ap()
    # last basis logits (cols 0:BN) + the n2 rows (cols BN:, rows 0:2K)
    tail_out = nc.dram_tensor("tail_out", [128, BN + BN], BF16, kind="ExternalOutput").ap()

    with tile.TileContext(nc) as tc:
        _emit(tc, big_in, fb_in, wam, lg_out, tail_out)
    nc.finalize()
    return nc


# Engine-assignment knobs (tuned against TimelineSim). pool (gpsimd) cannot
# read PSUM on real hw (BIR verifier rejects it) and DVE cannot read the same
# psum AP twice, so psum copies live on ACT/DVE and the sketch square always
# uses ACT's Square activation.
CFG = {
    "uv_copy": [("act", "dve"), ("dve", "act")],
    "uv_copy_last": ("act", "dve"),
    "lg": ["dve", "act", "dve", "act", "dve", "act", "dve", "act"],
    "lg_last": "act",
    "n2": "dve",
    "fused": ("dve", "act"),
    "warmup": 9,
    "warmup_ap": 256,
    "warmup_mid": 5,
    "z_prio": 60,
}


def _emit(tc, big_in, fb_in, wam, lg_out, tail_out):
    nc = tc.nc
    from contextlib import ExitStack

    def copy_psum(engine, dst, src):
        if engine == "dve":
            nc.vector.tensor_copy(dst, src)
        else:
            nc.scalar.activation(dst, src, AF.Copy)

    ctx = ExitStack()
    with ctx:
        const_pool = ctx.enter_context(tc.tile_pool(name="const", bufs=1))
        fused_pool = ctx.enter_context(tc.tile_pool(name="fused", bufs=1))
        w_pool = ctx.enter_context(tc.tile_pool(name="w", bufs=3))
        u_pool = ctx.enter_context(tc.tile_pool(name="u", bufs=2))
        v_pool = ctx.enter_context(tc.tile_pool(name="v", bufs=2))
        sq_pool = ctx.enter_context(tc.tile_pool(name="sq", bufs=2))
        lg_sb_pool = ctx.enter_context(tc.tile_pool(name="lgsb", bufs=4))
        tail_pool = ctx.enter_context(tc.tile_pool(name="tail", bufs=1))
        # 4 single-bank uv tiles + 1 z bank + 2 lg banks + 1 n2 bank = 8
        uv_ps = ctx.enter_context(tc.tile_pool(name="uv_ps", bufs=4, space="PSUM"))
        z_ps = ctx.enter_context(tc.tile_pool(name="z_ps", bufs=1, space="PSUM"))
        n2_ps = ctx.enter_context(tc.tile_pool(name="n2_ps", bufs=1, space="PSUM"))
        lg_ps = ctx.enter_context(tc.tile_pool(name="lg_ps", bufs=2, space="PSUM"))

        # PE warmup: junk matmuls hold the tensor engine busy through the
        # input-DMA prologue so the p-state ramp lands on junk. The operand
        # is uninitialized SBUF garbage (never read downstream; the warmup
        # psum rows are fully overwritten by the first start=True logits).
        # The one-column memset (allocates the tile) must be DVE's FIRST op.
        wmp = lg_ps.tile([128, BN], F32, tag="lg")
        wm_a = const_pool.tile([128, 256], FP8)
        nc.vector.memset(wm_a[:, 0:1], 0.0)

        # one-hot selector image for the norm-reduce matmuls: sliced
        # [128, 2, 16] windows all see ones at in-window col 4p + 2c +
        # (partition>=64) regardless of the pair index p.
        # chunk stride must be a multiple of 16B for dual-fp8 Ldweights
        oh = const_pool.tile([128, 32], FP8)
        nc.gpsimd.memset(oh[:], 0.0)
        nc.gpsimd.memset(oh[0:32, 12:13], 1.0)
        nc.gpsimd.memset(oh[32:64, 13:14], 1.0)
        nc.gpsimd.memset(oh[64:96, 14:15], 1.0)
        nc.gpsimd.memset(oh[96:128, 15:16], 1.0)

        fb_sb = const_pool.tile([1, D], BF16)
        ones_row = const_pool.tile([1, BN], BF16)
        nc.vector.memset(ones_row[:], 1.0)

        def junk_mm(ap):
            nc.tensor.matmul(
                wmp[0:64, 0:ap], wm_a[:, 0:64], wm_a[:, 0:ap],
                start=True, stop=True,
            )

        for _ in range(CFG["warmup"]):
            junk_mm(CFG["warmup_ap"])

        # all bases' sketch-norm sums accumulate into one [16, BN] psum bank
        # (row 2j = sum zq^2 of basis j, row 2j+1 = sum zk^2)
        n2_all = n2_ps.tile([2 * K, BN], F32, tag="n2")
        tail_sb = tail_pool.tile([128, 2 * BN], BF16)

        # --- fused = concat @ fw.T (+ fb), transposed, fp8 -----------------
        fusedT = fused_pool.tile([128, DC * BN], FP8)
        fusedT_v = fusedT.rearrange("p (d w) -> p d w", w=BN)
        PB = 2 * D + 2 * BN  # one interleaved pair-block: fw pair + concat pair
        with tc.tile_pool(name="prep", bufs=1) as prep_pool:
            big = prep_pool.tile([128, DC * PB], FP8)
            # three chunks: [pairs 0,1] then pair 2 and pair 3 separately,
            # so the c=2 round starts on its own completion sem
            for lo, hi in ((0, 2), (2, 3), (3, 4)):
                nc.sync.dma_start(
                    big[:, lo * PB : hi * PB],
                    big_in[:, lo * PB : hi * PB],
                )
                if lo == 0:
                    nc.sync.dma_start(fb_sb[:], fb_in)

            def fw_pair(c):
                return big[:, c * PB : c * PB + 2 * D].rearrange(
                    "p (c f) -> p c f", f=D
                )

            def conc_pair(c):
                return big[:, c * PB + 2 * D : (c + 1) * PB].rearrange(
                    "p (c w) -> p c w", w=BN
                )

            fps_t = [
                uv_ps.tile([128, BN], F32, tag="uv", name=f"fps{f}")
                for f in range(DC)
            ]
            fps = [t[:] for t in fps_t]
            # the bias rank-1 updates OPEN each accumulation group: they only
            # need fb/ones, so they run during the junk window before the
            # first concat chunk lands
            for f in range(DC):
                nc.tensor.matmul(
                    fps[f],
                    fb_sb[:, f * 128 : (f + 1) * 128],
                    ones_row[:],
                    start=True,
                    stop=False,
                )
            # first two c-rounds c-major (chunk A), last two f-major (chunk
            # B) so each f-bank's stop comes in copy order and the f0/f1
            # copies (which gate the first uv-proj) start 8 matmuls earlier
            for c in range(2):
                for f in range(DC):
                    nc.tensor.matmul(
                        fps[f],
                        fw_pair(c)[:, :, f * 128 : (f + 1) * 128],
                        conc_pair(c)[:],
                        start=False,
                        stop=False,
                        perf_mode=DR,
                    )
            for _ in range(CFG["warmup_mid"]):
                junk_mm(256)
            for f in range(DC):
                for c in range(2, DC):
                    nc.tensor.matmul(
                        fps[f],
                        fw_pair(c)[:, :, f * 128 : (f + 1) * 128],
                        conc_pair(c)[:],
                        start=False,
                        stop=(c == DC - 1),
                        perf_mode=DR,
                    )
            for f in range(DC):
                copy_psum(CFG["fused"][f % 2], fusedT_v[:, f, :], fps[f])

        # --- per-basis pipeline --------------------------------------------
        w_sbs = {}

        def load_w(j):
            w_sb = w_pool.tile([128, DC * WCOLS], FP8, tag="w")
            nc.sync.dma_start(w_sb[:], wam[j])
            w_sbs[j] = w_sb.rearrange("p (c w) -> p c w", w=WCOLS)

        def uvproj(j, u_sb, v_sb):
            # u = fused @ U128, v = fused @ (VS)128: separate single-bank
            # psum tiles and separate SBUF tiles so the two copies share no
            # tile (same-tile writers serialize in the scheduler)
            w_v = w_sbs[j]
            engs = CFG["uv_copy_last"] if j == K - 1 else CFG["uv_copy"][j % 2]
            for side, dst in ((0, u_sb), (1, v_sb)):
                ps = uv_ps.tile([128, BN], F32, tag="uv")
                for i in range(2):
                    nc.tensor.matmul(
                        ps[:],
                        w_v[:, 2 * i : 2 * i + 2,
                            side * RA : (side + 1) * RA],
                        fusedT_v[:, 2 * i : 2 * i + 2, :],
                        start=(i == 0),
                        stop=(i == 1),
                        perf_mode=DR,
                    )
                copy_psum(engs[side], dst[:], ps[:])

        def zproj(j, zp):
            # z = fused @ padded [Mq | Mk] (r=32): the even basis fills
            # sketch partitions 0:64, the odd one 64:128, via zero-padded
            # 128-wide stationaries accumulating into ONE psum group
            w_v = w_sbs[j]
            for i in range(2):
                nc.tensor.matmul(
                    zp[:],
                    w_v[:, 2 * i : 2 * i + 2, 2 * RA : 2 * RA + 128],
                    fusedT_v[:, 2 * i : 2 * i + 2, :],
                    start=(j % 2 == 0 and i == 0),
                    stop=(j % 2 == 1 and i == 1),
                    perf_mode=DR,
                    skip_group_check=True,
                )

        def ones_mm(pair, sq_tile):
            # one single-chunk matmul reduces the pair's four 32-partition
            # sketch zones into rows 4p..4p+3 of n2_all
            nc.tensor.matmul(
                n2_all[:],
                oh[:, 12 - 4 * pair : 28 - 4 * pair],
                sq_tile[:],
                start=(pair == 0),
                stop=(pair == K // 2 - 1),
                skip_group_check=True,
            )

        def logits(j, u_sb, v_sb):
            # lg[n, m] = sum_r u[r, n] v[r, m]: single-chunk contraction, one
            # matmul per local batch
            lg = lg_ps.tile([128, BN], F32, tag="lg")
            for b in range(BL):
                bs = slice(b * 128, (b + 1) * 128)
                nc.tensor.matmul(
                    lg[:, bs], u_sb[:, bs], v_sb[:, bs],
                    start=True, stop=True,
                )
            if j == K - 1:
                copy_psum(CFG["lg_last"], tail_sb[:, 0:BN], lg[:])
                nc.sync.dma_start(tail_out, tail_sb[:])
                return
            lgs = lg_sb_pool.tile([128, BN], BF16, tag="lgs")
            copy_psum(CFG["lg"][j], lgs[:], lg[:])
            nc.sync.dma_start(lg_out[j], lgs[:])

        load_w(0)
        load_w(1)
        prev = None
        sq_tile = None

        z_pair = None

        def z_work(j):
            # both bases of a pair project into ONE psum bank; one ACT
            # Square per pair halves the steady-state ACT load
            nonlocal z_pair, sq_tile
            if j % 2 == 0:
                z_pair = z_ps.tile([128, BN], F32, tag="z")
            zproj(j, z_pair)
            if j % 2 == 1:
                sq_tile = sq_pool.tile([128, BN], FP8, tag="sq")
                nc.scalar.activation(sq_tile[:], z_pair[:], AF.Square)
                ones_mm(j // 2, sq_tile)

        for j in range(K - 2):
            load_w(j + 2)
            u_sb = u_pool.tile([128, BN], FP8, tag="u")
            v_sb = v_pool.tile([128, BN], FP8, tag="v")
            uvproj(j, u_sb, v_sb)
            if prev is not None:
                logits(prev[2], prev[0], prev[1])
            z_work(j)
            prev = (u_sb, v_sb, j)
            del w_sbs[j]

        # last two bases: both uv projections run back-to-back so the
        # u7/v7 copies (which gate the final logits) start a cycle early
        u6 = u_pool.tile([128, BN], FP8, tag="u")
        v6 = v_pool.tile([128, BN], FP8, tag="v")
        u7 = u_pool.tile([128, BN], FP8, tag="u", name="u7")
        v7 = v_pool.tile([128, BN], FP8, tag="v", name="v7")
        uvproj(K - 2, u6, v6)
        uvproj(K - 1, u7, v7)
        logits(prev[2], prev[0], prev[1])
        z_work(K - 2)
        with tc.high_priority(offset=CFG["z_prio"]):
            z_work(K - 1)
            copy_psum(CFG["n2"], tail_sb[0 : 2 * K, BN:], n2_all[:])
        logits(K - 2, u6, v6)
        logits(K - 1, u7, v7)


_CACHE = {}


def _get_nc():
    if "nc" not in _CACHE:
        _CACHE["nc"] = build_kernel()
    return _CACHE["nc"]


def shard_inputs(desc_embeddings, name_value_embeddings, W_q, W_k, fusion_w, fusion_b):
    import ml_dtypes

    fp8 = ml_dtypes.float8_e4m3
    big = np.concatenate(
        [np.asarray(desc_embeddings, np.float32), np.asarray(name_value_embeddings, np.float32)],
        axis=-1,
    )  # [B, N, 2D]
    fwt = (
        np.asarray(fusion_w, np.float32).T.reshape(CC, 128, D)
        .transpose(1, 0, 2).reshape(128, CC * D).astype(fp8)
    )
    fb_row = np.ascontiguousarray(
        np.asarray(fusion_b, np.float32)[None, :].astype(ml_dtypes.bfloat16)
    )

    W_q = np.asarray(W_q, np.float32)
    W_k = np.asarray(W_k, np.float32)
    Pq = np.random.default_rng(1).standard_normal((R, D)).astype(np.float32)
    Pk = np.random.default_rng(2).standard_normal((R, D)).astype(np.float32)
    wam = np.empty((K, 128, DC * WCOLS), dtype=fp8)
    for k in range(K):
        A = W_q[k] @ W_k[k].T
        U, S, Vt = np.linalg.svd(A)
        Ur = U[:, :RA] * U_SCALE                             # [D, RA]
        Vr = Vt[:RA].T * S[:RA] * V_SCALE                    # [D, RA]
        Mq = (W_q[k] @ Pq.T) * Z_SCALE                       # [D, R]
        Mk = (W_k[k] @ Pk.T) * Z_SCALE
        pad = np.zeros((D, 64), np.float32)
        m_blk = (np.concatenate([Mq, Mk, pad], axis=1) if k % 2 == 0
                 else np.concatenate([pad, Mq, Mk], axis=1))  # [D, 128]
        blk = np.concatenate([Ur, Vr, m_blk], axis=1)        # [D, WCOLS]
        wam[k] = blk.reshape(DC, 128, WCOLS).transpose(1, 0, 2).reshape(
            128, DC * WCOLS).astype(fp8)

    full = {"fb_in": fb_row, "wam": wam}
    in_maps = []
    for c in range(CORES):
        x = big[c * BL : (c + 1) * BL]  # [BL, N, 2D]
        img = (
            x.transpose(2, 0, 1).reshape(CC, 128, BL * N)
            .transpose(1, 0, 2).reshape(128, CC * BN).astype(fp8)
        )
        blocks = []
        for cc in range(DC):
            blocks.append(fwt[:, cc * 2 * D : (cc + 1) * 2 * D])
            blocks.append(img[:, cc * 2 * BN : (cc + 1) * 2 * BN])
        m = dict(full)
        m["big_in"] = np.ascontiguousarray(np.concatenate(blocks, axis=1))
        in_maps.append(m)
    return in_maps


def assemble_outputs(results):
    tail = np.stack([np.asarray(r["tail_out"], np.float32) for r in results])
    lg = np.stack([np.asarray(r["lg_out"], np.float32) for r in results])
    lg = np.concatenate([lg, tail[:, None, :, 0:BN]], axis=1)  # [C, K, N, BN]
    n2 = tail[:, 0 : 2 * K, BN:]                               # [C, 2K, BN]
    # [C, K, N, BL*N] -> [B, K, N, N]
    lg = lg.reshape(CORES, K, N, BL, N).transpose(0, 3, 1, 2, 4).reshape(B, K, N, N)
    # n2 rows 2j/2j+1 = sum (z/16)^2 = |z|^2/256; n2_hat = |z|^2/R
    n2 = n2.reshape(CORES, K, 2, BL, N) * (1.0 / (Z_SCALE * Z_SCALE * R))
    inv = 1.0 / np.sqrt(np.maximum(n2, 1e-24))              # [C, K, 2, BL, N]
    invq = inv[:, :, 0].transpose(0, 2, 1, 3).reshape(B, K, N)
    invk = inv[:, :, 1].transpose(0, 2, 1, 3).reshape(B, K, N)
    logits = lg * invq[:, :, :, None] * invk[:, :, None, :] * (
        1.0 / (U_SCALE * V_SCALE * np.sqrt(D))
    )
    mx = logits.max(-1, keepdims=True)
    e = np.exp(logits - mx)
    alpha = (e / e.sum(-1, keepdims=True)).astype(np.float32)
    ema = np.float32(1.0 - MOMENTUM) * alpha.mean(0)
    bias_log = np.log(np.maximum(ema, np.float32(EPS)))
    bias_log = np.broadcast_to(bias_log[None], (B, K, N, N)).astype(np.float32)
    return bias_log, alpha


def kernel(desc_embeddings, name_value_embeddings, W_q, W_k, fusion_w, fusion_b,
           _trace=False):
    nc = _get_nc()
    in_maps = shard_inputs(
        desc_embeddings, name_value_embeddings, W_q, W_k, fusion_w, fusion_b
    )
    res = run_bass_kernel_spmd(nc, in_maps, core_ids=list(range(CORES)), trace=_trace)
    out = assemble_outputs(res.results)
    if _trace:
        return out, res
    return out
